# revision 20
# baseline (speedup 1.0000x reference)
"""GAT (2-layer, PyG-style) on 8 Trainium2 NeuronCores.

Strategy (edge parallelism per the sharding hint), v3 — three launches:
  - Nodes are split into 8 contiguous ranges (12500/core); each core owns all
    in-edges of its nodes (~412K edges, uniform since the graph is random).
    Per-core nodes are degree-sorted into 128-lane tiles. Tiles are packed
    into groups with a UNIFORM slot depth Dg per group (pad slots reference a
    sentinel row whose a_src = -30000 so e underflows to exactly 0; ~3.5%
    padding thanks to the degree sort).
  - Launch A (node pass): R1 = [h1 | a_src1 | a_dst1] = x @ [W1|W1@As|W1@Ad]
    once per node on the PE (fp16, weights stationary, channel-major out).
  - Host gathers R1[src] per edge slot (72B/edge fp16; 3.5x less HBM than
    raw-x gather, no per-edge matmuls).
  - Launch B (layer-1 edge pass): per group, stream channel-major planes
    [h(32) | a_src(2) | a_dst(2)] fp16; e = exp(lrelu(a_src+a_dst)) (exp on
    ACT); V = e*h with one big packed-fp16 DVE mult (fast path ~0.19ns/elem);
    segment-sum via IN-PLACE fp16 halving-tree adds over the uniform slot
    axis (elementwise fp16 is ~6x faster than tensor_reduce on HW), final
    pair summed to f32. Channel-major finishing: normalize, +b1, ELU,
    R2 = eluT @ [W2|W2@As2|W2@Ad2] via PE transposes.
  - Host gathers R2[src] per edge slot (8B/edge); launch C does layer 2 the
    same way; with only 2 output channels the softmax/log_softmax needs no
    reduces at all (channel-pair tensor_tensor ops).
"""

import sys

sys.path.insert(0, "/opt/trn_rl_repo")

from contextlib import ExitStack

import numpy as np

import concourse.tile as tile
from concourse import bass, mybir
from concourse.bass_utils import run_bass_kernel_spmd
from concourse.masks import make_identity

F32 = mybir.dt.float32
F16 = mybir.dt.float16
NP16 = np.float16

NC = 8
TILE = 128
NH = 2
CH = 16
D1 = NH * CH  # 32
REC = D1 + 2 * NH  # 36
NEG_SLOPE = 0.2
BIG_NEG = -30000.0  # fp16-safe; 0.2*BIG_NEG underflows exp to exactly 0
L_BUDGET = 384  # max (padded) slot columns per group


_ws_seq = [0]


def _split_waits(nc, limit=1):
    """The walrus build in this container rejects instructions carrying more
    than one sem wait ("Too many sync wait commands"). Hoist excess waits
    onto NOP carriers inserted just before the instruction (same engine, same
    program order, so semantics are preserved)."""
    for f in nc.m.functions:
        for blk in f.blocks:
            il = list(blk.instructions)
            out = []
            changed = False
            for inst in il:
                si = inst.sync_info
                waits = list(si.on_wait) if (si and si.on_wait) else []
                if len(waits) > limit:
                    keep = waits[-limit:]
                    for w in waits[:-limit]:
                        _ws_seq[0] += 1
                        nop = mybir.InstNoOp(name=f"WS-{_ws_seq[0]}")
                        nop.engine = inst.engine
                        nop.sync_info = mybir.SyncInfo(on_wait=[w], on_update=[])
                        out.append(nop)
                    si.on_wait = keep
                    changed = True
                out.append(inst)
            if changed:
                blk.instructions = out


# ---------------------------------------------------------------- host prep


def _plan(src, dst, n_nodes, n_cores):
    """Node ranges, degree-sorted tiles, shared D_t schedule, slot src ids."""
    per = n_nodes // n_cores
    ntiles = (per + TILE - 1) // TILE
    padn = ntiles * TILE

    deg = np.bincount(dst, minlength=n_nodes)

    # edges sorted by dst, self-loop (src==dst) first within each segment
    order_e = np.lexsort((src != dst, dst))
    s_src = src[order_e]
    rowptr = np.zeros(n_nodes + 1, dtype=np.int64)
    np.cumsum(deg, out=rowptr[1:])

    orders = []  # per core: global node id per sorted slot lane (-1 = fake)
    Dt_all = np.zeros((n_cores, ntiles), dtype=np.int64)
    for c in range(n_cores):
        d = deg[c * per : (c + 1) * per]
        ids = np.concatenate(
            [c * per + np.arange(per), np.full(padn - per, -1, np.int64)]
        )
        dd = np.concatenate([d, np.zeros(padn - per, np.int64)])
        o = np.argsort(dd, kind="stable")
        orders.append(ids[o])
        Dt_all[c] = dd[o].reshape(ntiles, TILE).max(axis=1)
    Dt = Dt_all.max(axis=0)
    Dt = np.maximum(Dt, 1)  # avoid zero-size tiles
    nblocks = int(Dt.sum())

    # slot src ids per core: [nblocks, TILE] int64, pad = n_nodes
    slot_src = np.full((n_cores, nblocks, TILE), n_nodes, dtype=np.int64)
    for c in range(n_cores):
        ids = orders[c]
        b0 = 0
        for t in range(ntiles):
            D = int(Dt[t])
            nid = ids[t * TILE : (t + 1) * TILE]
            real = nid >= 0
            nid_c = np.where(real, nid, 0)
            degs = np.where(real, deg[nid_c], 0)
            jj = np.arange(D)[:, None]  # [D, TILE]
            valid = jj < degs[None, :]
            eidx = rowptr[nid_c][None, :] + np.minimum(jj, np.maximum(degs - 1, 0))
            vals = s_src[np.clip(eidx, 0, len(s_src) - 1)]
            slot_src[c, b0 : b0 + D] = np.where(valid, vals, n_nodes)
            b0 += D
    return per, ntiles, padn, Dt, nblocks, slot_src, orders


def _groups(Dt):
    """Pack degree-sorted tiles into groups with a UNIFORM padded slot depth.

    Returns [(t0, ng, Dg)]: tiles t0..t0+ng-1, each padded to Dg slots."""
    groups = []
    t0 = 0
    for t in range(len(Dt)):
        if (t - t0 + 1) * int(Dt[t]) > L_BUDGET and t > t0:
            groups.append((t0, t - t0, int(Dt[t - 1])))
            t0 = t
    groups.append((t0, len(Dt) - t0, int(Dt[-1])))
    return groups


def _padded_slots(slot_src_c, Dt, groups, n_nodes):
    """Per-group padded slot-src tables: [L_g, TILE] with sentinel pads."""
    out = []
    blk = 0
    for t0, ng, dg in groups:
        tab = np.full((ng, dg, TILE), n_nodes, dtype=np.int64)
        for i in range(ng):
            D = int(Dt[t0 + i])
            tab[i, 0:D] = slot_src_c[blk : blk + D]
            blk += D
        out.append(tab.reshape(ng * dg, TILE))
    return out


# ------------------------------------------------------- launch A (node pass)


def _build_a(padn, fdim, repeat=None):
    """R1 = [h1 | a_src1 | a_dst1] = w1p.T @ x, channel-major out [REC, padn]."""
    nc = bass.Bass("TRN2")
    xt = nc.declare_dram_parameter("xt", [fdim, padn], F16, isOutput=False)
    w1p = nc.declare_dram_parameter("w1p", [fdim, REC], F16, isOutput=False)
    r1 = nc.declare_dram_parameter("r1", [REC, padn], F16, isOutput=True)
    nt = padn // TILE

    with ExitStack() as ctx:
        tc = ctx.enter_context(tile.TileContext(nc))
        const = ctx.enter_context(tc.tile_pool(name="const", bufs=1))
        xp = ctx.enter_context(tc.tile_pool(name="xp", bufs=1))
        pp = ctx.enter_context(tc.tile_pool(name="pp", bufs=4, space="PSUM"))
        op = ctx.enter_context(tc.tile_pool(name="op", bufs=1))

        w1t = const.tile([fdim, REC], F16)
        nc.sync.dma_start(out=w1t[:], in_=w1p[:])

        if repeat:
            ctx.enter_context(tc.For_i(0, repeat, 1))
        xtile = xp.tile([fdim, padn], F16, tag="xt")
        nc.sync.dma_start(out=xtile[:], in_=xt[:])
        r1sb = op.tile([REC, padn], F16, tag="r1sb")
        p1 = None
        for t in range(nt):
            q = t % 4
            if q == 0:
                p1 = pp.tile([REC, 4 * TILE], F32, tag="p1")
            nc.tensor.matmul(
                out=p1[:, q * TILE : (q + 1) * TILE],
                lhsT=w1t[:],
                rhs=xtile[:, t * TILE : (t + 1) * TILE],
                start=True,
                stop=True,
            )
            if q == 3 or t == nt - 1:
                nc.vector.tensor_copy(
                    out=r1sb[:, (t - q) * TILE : (t + 1) * TILE],
                    in_=p1[:, 0 : (q + 1) * TILE],
                )
        nc.sync.dma_start(out=r1[:], in_=r1sb[:])
    return nc


# ------------------------------------------------------------- launch B (L1)


def _build_l1(Dt, groups, ntiles, padn, repeat=None):
    """Layer-1 edge pass from host-gathered channel-major slot planes.

    Group stages are issued PAIR-INTERLEAVED: on this HW a DVE instruction
    that waits on the immediately preceding instruction stalls ~2.7us, and
    the in-order queue serializes the stalls; alternating two independent
    groups' instructions hides the dependency latency. Segment-sum = one
    fp16 halving level + one fp16->fp16 tensor_reduce per group."""
    cb = 36 * sum(ng * dg for _, ng, dg in groups)
    lgm = max(ng * dg for _, ng, dg in groups)
    caph = D1 * max(ng * (dg // 2 + dg % 2) for _, ng, dg in groups)
    nc = bass.Bass("TRN2")
    hsd = nc.declare_dram_parameter("hsd", [TILE, cb], F16, isOutput=False)
    b1r = nc.declare_dram_parameter("b1r", [TILE, D1], F16, isOutput=False)
    w2p = nc.declare_dram_parameter("w2p", [D1, 4], F16, isOutput=False)
    r2 = nc.declare_dram_parameter("r2", [padn, 4], F16, isOutput=True)

    offs = []
    off = 0
    for _, ng, dg in groups:
        offs.append(off)
        off += 36 * ng * dg

    with ExitStack() as ctx:
        tc = ctx.enter_context(tile.TileContext(nc))
        const = ctx.enter_context(tc.tile_pool(name="const", bufs=1))
        hspool = ctx.enter_context(tc.tile_pool(name="hs", bufs=2))
        wk = ctx.enter_context(tc.tile_pool(name="wk", bufs=2))
        vpool = ctx.enter_context(tc.tile_pool(name="vp", bufs=2))
        lvl = ctx.enter_context(tc.tile_pool(name="lvl", bufs=2))
        ppool = ctx.enter_context(tc.tile_pool(name="pp", bufs=2, space="PSUM"))
        rpool = ctx.enter_context(tc.tile_pool(name="rp", bufs=2, space="PSUM"))
        outp = ctx.enter_context(tc.tile_pool(name="op", bufs=1))

        b1t = const.tile([TILE, D1], F16)
        nc.sync.dma_start(out=b1t[:], in_=b1r[:])
        w2t = const.tile([D1, 4], F16)
        nc.sync.dma_start(out=w2t[:], in_=w2p[:])
        ident = const.tile([TILE, TILE], F16)
        make_identity(nc, ident[:])

        if repeat:
            ctx.enter_context(tc.For_i(0, repeat, 1))
        o1a16 = outp.tile([TILE, D1, ntiles], F16, tag="o1a16")
        sa16 = outp.tile([TILE, NH, ntiles], F16, tag="sa16")

        st = {}
        pairs = [groups[i : i + 2] for i in range(0, len(groups), 2)]
        gidx = 0
        for pair in pairs:
            ids = list(range(gidx, gidx + len(pair)))
            gidx += len(pair)
            for gi, (t0, ng, dg) in zip(ids, pair):
                L = ng * dg
                hst = hspool.tile([TILE, 36, lgm], F16, tag="hst")
                nc.sync.dma_start(
                    out=hst[:, :, 0:L],
                    in_=hsd[:, offs[gi] : offs[gi] + 36 * L].rearrange(
                        "p (c j) -> p c j", c=36
                    ),
                )
                st[gi] = {"hst": hst}
            for gi, (t0, ng, dg) in zip(ids, pair):
                L = ng * dg
                lg = wk.tile([TILE, NH, lgm], F16, tag="lg")
                nc.vector.tensor_tensor(
                    out=lg[:, :, 0:L],
                    in0=st[gi]["hst"][:, D1 : D1 + NH, 0:L],
                    in1=st[gi]["hst"][:, D1 + NH : REC, 0:L],
                    op=mybir.AluOpType.add,
                )
                st[gi]["lg"] = lg
            for gi, (t0, ng, dg) in zip(ids, pair):
                L = ng * dg
                ls = wk.tile([TILE, NH, lgm], F16, tag="ls")
                nc.vector.tensor_scalar_mul(
                    out=ls[:, :, 0:L], in0=st[gi]["lg"][:, :, 0:L],
                    scalar1=NEG_SLOPE,
                )
                st[gi]["ls"] = ls
            for gi, (t0, ng, dg) in zip(ids, pair):
                L = ng * dg
                nc.vector.tensor_tensor(
                    out=st[gi]["lg"][:, :, 0:L],
                    in0=st[gi]["lg"][:, :, 0:L],
                    in1=st[gi]["ls"][:, :, 0:L],
                    op=mybir.AluOpType.max,
                )
            for gi, (t0, ng, dg) in zip(ids, pair):
                L = ng * dg
                et = wk.tile([TILE, NH, lgm], F16, tag="et")
                nc.scalar.activation(
                    out=et[:, :, 0:L], in_=st[gi]["lg"][:, :, 0:L],
                    func=mybir.ActivationFunctionType.Exp,
                )
                st[gi]["et"] = et
            for gi, (t0, ng, dg) in zip(ids, pair):
                L = ng * dg
                V = vpool.tile([TILE, NH, CH, lgm], F16, tag="V")
                nc.vector.tensor_tensor(
                    out=V[:, :, :, 0:L],
                    in0=st[gi]["hst"][:, 0:D1, 0:L].rearrange(
                        "p (h c) j -> p h c j", h=NH
                    ),
                    in1=st[gi]["et"][:, :, 0:L]
                    .unsqueeze(2)
                    .to_broadcast([TILE, NH, CH, L]),
                    op=mybir.AluOpType.mult,
                )
                st[gi]["V"] = V
            for gi, (t0, ng, dg) in zip(ids, pair):
                # halving level: Vh[.., j] = V[.., j] + V[.., k+j]
                L = ng * dg
                k = dg // 2
                odd = dg % 2
                hv = k + odd
                flat = lvl.tile([TILE, caph], F16, tag="lv")
                Vh = flat[:, 0 : D1 * ng * hv].rearrange(
                    "p (c t j) -> p c t j", c=D1, t=ng
                )
                V5 = st[gi]["V"][:, :, :, 0:L].rearrange(
                    "p h c (t j) -> p (h c) t j", j=dg
                )
                nc.vector.tensor_tensor(
                    out=Vh[:, :, :, 0:k],
                    in0=V5[:, :, :, 0:k],
                    in1=V5[:, :, :, k : 2 * k],
                    op=mybir.AluOpType.add,
                )
                if odd:
                    nc.vector.tensor_copy(
                        out=Vh[:, :, :, k : k + 1],
                        in_=V5[:, :, :, 2 * k : dg],
                    )
                st[gi]["Vh"] = Vh
                st[gi]["hv"] = hv
            with nc.allow_low_precision(reason="fp16 segment sums, 2e-2 gate"):
                for gi, (t0, ng, dg) in zip(ids, pair):
                    nc.vector.tensor_reduce(
                        out=o1a16[:, :, t0 : t0 + ng],
                        in_=st[gi]["Vh"],
                        axis=mybir.AxisListType.X,
                        op=mybir.AluOpType.add,
                    )
                for gi, (t0, ng, dg) in zip(ids, pair):
                    L = ng * dg
                    E5 = st[gi]["et"][:, :, 0:L].rearrange(
                        "p h (t j) -> p h t j", j=dg
                    )
                    nc.vector.tensor_reduce(
                        out=sa16[:, :, t0 : t0 + ng],
                        in_=E5,
                        axis=mybir.AxisListType.X,
                        op=mybir.AluOpType.add,
                    )
            st.clear()

        # ---- finishing: two tile-range halves, instruction-interleaved ----
        inv = outp.tile([TILE, NH, ntiles], F32, tag="inv")
        invh = outp.tile([TILE, NH, ntiles], F16, tag="invh")
        o1f = outp.tile([TILE, NH, CH, ntiles], F16, tag="o1f")
        e1 = outp.tile([TILE, NH, CH, ntiles], F16, tag="e1")

        def fin_chain(ts, te):
            w = te - ts
            yield lambda: nc.vector.tensor_scalar_add(
                out=inv[:, :, ts:te], in0=sa16[:, :, ts:te], scalar1=1e-16
            )
            yield lambda: nc.vector.reciprocal(
                out=inv[:, :, ts:te], in_=inv[:, :, ts:te]
            )
            yield lambda: nc.vector.tensor_copy(
                out=invh[:, :, ts:te], in_=inv[:, :, ts:te]
            )
            yield lambda: nc.vector.tensor_tensor(
                out=o1f[:, :, :, ts:te],
                in0=o1a16[:, :, ts:te].rearrange("p (h c) t -> p h c t", h=NH),
                in1=invh[:, :, ts:te]
                .unsqueeze(2)
                .to_broadcast([TILE, NH, CH, w]),
                op=mybir.AluOpType.mult,
            )
            yield lambda: nc.vector.tensor_tensor(
                out=o1f[:, :, :, ts:te],
                in0=o1f[:, :, :, ts:te],
                in1=b1t[:]
                .rearrange("p (h c) -> p h c", h=NH)
                .unsqueeze(-1)
                .to_broadcast([TILE, NH, CH, w]),
                op=mybir.AluOpType.add,
            )
            yield lambda: nc.vector.tensor_scalar_min(
                out=e1[:, :, :, ts:te], in0=o1f[:, :, :, ts:te], scalar1=0.0
            )
            yield lambda: nc.scalar.activation(
                out=e1[:, :, :, ts:te],
                in_=e1[:, :, :, ts:te],
                func=mybir.ActivationFunctionType.Exp,
            )
            yield lambda: nc.vector.tensor_scalar_add(
                out=e1[:, :, :, ts:te], in0=e1[:, :, :, ts:te], scalar1=-1.0
            )
            yield lambda: nc.vector.tensor_scalar_max(
                out=o1f[:, :, :, ts:te], in0=o1f[:, :, :, ts:te], scalar1=0.0
            )
            yield lambda: nc.vector.tensor_tensor(
                out=o1f[:, :, :, ts:te],
                in0=o1f[:, :, :, ts:te],
                in1=e1[:, :, :, ts:te],
                op=mybir.AluOpType.add,
            )

        half = ntiles // 2
        for s1, s2 in zip(fin_chain(0, half), fin_chain(half, ntiles)):
            s1()
            s2()

        # R2 = [h2 | a_src2 | a_dst2] = elu_out @ w2p via PE transposes
        o1tsb = outp.tile([D1, padn], F16, tag="o1t")
        pt = None
        for t in range(ntiles):
            q = t % 4
            if q == 0:
                pt = ppool.tile([D1, 4 * TILE], F16, tag="pt")
            nc.tensor.transpose(
                out=pt[:, q * TILE : (q + 1) * TILE],
                in_=o1f[:, :, :, t].rearrange("p h c -> p (h c)"),
                identity=ident[:],
            )
            if q == 3 or t == ntiles - 1:
                nc.vector.tensor_copy(
                    out=o1tsb[:, (t - q) * TILE : (t + 1) * TILE],
                    in_=pt[:, 0 : (q + 1) * TILE],
                )
        r2all = outp.tile([TILE, ntiles, 4], F16, tag="r2all")
        r2p = None
        for t in range(ntiles):
            q = t % 32
            if q == 0:
                r2p = rpool.tile([TILE, 32 * 4], F32, tag="r2p")
            nc.tensor.matmul(
                out=r2p[:, q * 4 : (q + 1) * 4],
                lhsT=o1tsb[:, t * TILE : (t + 1) * TILE],
                rhs=w2t[:],
                start=True,
                stop=True,
            )
            if q == 31 or t == ntiles - 1:
                nc.vector.tensor_copy(
                    out=r2all[:, t - q : t + 1, :],
                    in_=r2p[:, 0 : (q + 1) * 4].rearrange("p (t c) -> p t c", c=4),
                )
        nc.sync.dma_start(
            out=r2[:].rearrange("(t n) c -> n t c", n=TILE), in_=r2all[:]
        )
    return nc


# ------------------------------------------------------------- launch C (L2)


def _build_l2(Dt, groups, ntiles, padn, repeat=None):
    """Layer 2 (1 head, 2 ch) from grouped planar [h2(2) | a_src2 | a_dst2]
    slots, bias and log_softmax (channel-pair ops, no reduces in finishing).
    Group stages pair-interleaved as in _build_l1."""
    cb = 4 * sum(ng * dg for _, ng, dg in groups)
    lgm = max(ng * dg for _, ng, dg in groups)
    nc = bass.Bass("TRN2")
    xed = nc.declare_dram_parameter("xed", [TILE, cb], F16, isOutput=False)
    b2r = nc.declare_dram_parameter("b2r", [TILE, 2], F32, isOutput=False)
    y = nc.declare_dram_parameter("y", [padn, 2], F32, isOutput=True)

    offs = []
    off = 0
    for _, ng, dg in groups:
        offs.append(off)
        off += 4 * ng * dg

    with ExitStack() as ctx:
        tc = ctx.enter_context(tile.TileContext(nc))
        const = ctx.enter_context(tc.tile_pool(name="const", bufs=1))
        xp = ctx.enter_context(tc.tile_pool(name="xp", bufs=2))
        wk = ctx.enter_context(tc.tile_pool(name="wk", bufs=2))
        outp = ctx.enter_context(tc.tile_pool(name="op", bufs=1))

        b2t = const.tile([TILE, 2], F32)
        nc.sync.dma_start(out=b2t[:], in_=b2r[:])

        if repeat:
            ctx.enter_context(tc.For_i(0, repeat, 1))
        acc2 = outp.tile([TILE, 2, ntiles], F16, tag="acc2")
        s2h = outp.tile([TILE, ntiles], F16, tag="s2h")

        st = {}
        pairs = [groups[i : i + 2] for i in range(0, len(groups), 2)]
        gidx = 0
        for pair in pairs:
            ids = list(range(gidx, gidx + len(pair)))
            gidx += len(pair)
            for gi, (t0, ng, dg) in zip(ids, pair):
                L = ng * dg
                xe = xp.tile([TILE, 4, lgm], F16, tag="xe")
                nc.sync.dma_start(
                    out=xe[:, :, 0:L],
                    in_=xed[:, offs[gi] : offs[gi] + 4 * L].rearrange(
                        "p (c j) -> p c j", c=4
                    ),
                )
                st[gi] = {"xe": xe}
            for gi, (t0, ng, dg) in zip(ids, pair):
                L = ng * dg
                lg = wk.tile([TILE, lgm], F16, tag="lg")
                nc.vector.tensor_tensor(
                    out=lg[:, 0:L], in0=st[gi]["xe"][:, 2, 0:L],
                    in1=st[gi]["xe"][:, 3, 0:L], op=mybir.AluOpType.add,
                )
                st[gi]["lg"] = lg
            for gi, (t0, ng, dg) in zip(ids, pair):
                L = ng * dg
                ls = wk.tile([TILE, lgm], F16, tag="ls")
                nc.vector.tensor_scalar_mul(
                    out=ls[:, 0:L], in0=st[gi]["lg"][:, 0:L], scalar1=NEG_SLOPE
                )
                st[gi]["ls"] = ls
            for gi, (t0, ng, dg) in zip(ids, pair):
                L = ng * dg
                nc.vector.tensor_tensor(
                    out=st[gi]["lg"][:, 0:L], in0=st[gi]["lg"][:, 0:L],
                    in1=st[gi]["ls"][:, 0:L], op=mybir.AluOpType.max,
                )
            for gi, (t0, ng, dg) in zip(ids, pair):
                L = ng * dg
                et = wk.tile([TILE, lgm], F16, tag="et")
                nc.scalar.activation(
                    out=et[:, 0:L], in_=st[gi]["lg"][:, 0:L],
                    func=mybir.ActivationFunctionType.Exp,
                )
                st[gi]["et"] = et
            for gi, (t0, ng, dg) in zip(ids, pair):
                L = ng * dg
                V = wk.tile([TILE, 2, lgm], F16, tag="V")
                nc.vector.tensor_tensor(
                    out=V[:, :, 0:L],
                    in0=st[gi]["xe"][:, 0:2, 0:L],
                    in1=st[gi]["et"][:, 0:L]
                    .unsqueeze(1)
                    .to_broadcast([TILE, 2, L]),
                    op=mybir.AluOpType.mult,
                )
                st[gi]["V"] = V
            with nc.allow_low_precision(reason="fp16 segment sums, 2e-2 gate"):
                for gi, (t0, ng, dg) in zip(ids, pair):
                    L = ng * dg
                    V5 = st[gi]["V"][:, :, 0:L].rearrange(
                        "p c (t j) -> p c t j", j=dg
                    )
                    nc.vector.tensor_reduce(
                        out=acc2[:, :, t0 : t0 + ng],
                        in_=V5,
                        axis=mybir.AxisListType.X,
                        op=mybir.AluOpType.add,
                    )
                for gi, (t0, ng, dg) in zip(ids, pair):
                    L = ng * dg
                    E5 = st[gi]["et"][:, 0:L].rearrange(
                        "p (t j) -> p t j", j=dg
                    )
                    nc.vector.tensor_reduce(
                        out=s2h[:, t0 : t0 + ng],
                        in_=E5,
                        axis=mybir.AxisListType.X,
                        op=mybir.AluOpType.add,
                    )
            st.clear()

        # ---- finishing: two tile-range halves, instruction-interleaved ----
        inv = outp.tile([TILE, ntiles], F32, tag="inv")
        z = outp.tile([TILE, 2, ntiles], F32, tag="z")
        m = outp.tile([TILE, ntiles], F32, tag="m")
        ez = outp.tile([TILE, 2, ntiles], F32, tag="ez")
        ss = outp.tile([TILE, ntiles], F32, tag="ss")
        yt = outp.tile([TILE, 2, ntiles], F32, tag="yt")
        yt2 = outp.tile([TILE, ntiles, 2], F32, tag="yt2")

        def fin_chain(ts, te):
            w = te - ts
            yield lambda: nc.vector.tensor_scalar_add(
                out=inv[:, ts:te], in0=s2h[:, ts:te], scalar1=1e-16
            )
            yield lambda: nc.vector.reciprocal(
                out=inv[:, ts:te], in_=inv[:, ts:te]
            )
            yield lambda: nc.vector.tensor_tensor(
                out=z[:, :, ts:te],
                in0=acc2[:, :, ts:te],
                in1=inv[:, ts:te].unsqueeze(1).to_broadcast([TILE, 2, w]),
                op=mybir.AluOpType.mult,
            )
            yield lambda: nc.vector.tensor_tensor(
                out=z[:, :, ts:te],
                in0=z[:, :, ts:te],
                in1=b2t[:].unsqueeze(-1).to_broadcast([TILE, 2, w]),
                op=mybir.AluOpType.add,
            )
            yield lambda: nc.vector.tensor_tensor(
                out=m[:, ts:te], in0=z[:, 0, ts:te], in1=z[:, 1, ts:te],
                op=mybir.AluOpType.max,
            )
            yield lambda: nc.vector.tensor_tensor(
                out=z[:, :, ts:te],
                in0=z[:, :, ts:te],
                in1=m[:, ts:te].unsqueeze(1).to_broadcast([TILE, 2, w]),
                op=mybir.AluOpType.subtract,
            )
            yield lambda: nc.scalar.activation(
                out=ez[:, :, ts:te], in_=z[:, :, ts:te],
                func=mybir.ActivationFunctionType.Exp,
            )
            yield lambda: nc.vector.tensor_tensor(
                out=ss[:, ts:te], in0=ez[:, 0, ts:te], in1=ez[:, 1, ts:te],
                op=mybir.AluOpType.add,
            )
            yield lambda: nc.scalar.activation(
                out=ss[:, ts:te], in_=ss[:, ts:te],
                func=mybir.ActivationFunctionType.Ln,
            )
            yield lambda: nc.vector.tensor_tensor(
                out=yt[:, :, ts:te],
                in0=z[:, :, ts:te],
                in1=ss[:, ts:te].unsqueeze(1).to_broadcast([TILE, 2, w]),
                op=mybir.AluOpType.subtract,
            )
            yield lambda: nc.vector.tensor_copy(
                out=yt2[:, ts:te, :],
                in_=yt[:, :, ts:te].rearrange("p c t -> p t c"),
            )

        half = ntiles // 2
        for s1, s2 in zip(fin_chain(0, half), fin_chain(half, ntiles)):
            s1()
            s2()
        nc.sync.dma_start(
            out=y[:].rearrange("(t n) c -> n t c", n=TILE), in_=yt2[:]
        )
    return nc


# ------------------------------------------------------------------- driver


def _run_gat(x, edge_index, W1, att_src1, att_dst1, b1, W2, att_src2, att_dst2, b2,
             n_cores=NC, timing=None):
    n_nodes, fdim = x.shape
    nh, ch = att_src1.shape

    src = np.concatenate([np.asarray(edge_index[0]), np.arange(n_nodes)]).astype(
        np.int64
    )
    dst = np.concatenate([np.asarray(edge_index[1]), np.arange(n_nodes)]).astype(
        np.int64
    )

    per, ntiles, padn, Dt, nblocks, slot_src, orders = _plan(
        src, dst, n_nodes, n_cores
    )
    groups = _groups(Dt)

    W1 = np.asarray(W1, np.float32)
    att_src1 = np.asarray(att_src1, np.float32)
    att_dst1 = np.asarray(att_dst1, np.float32)
    W2 = np.asarray(W2, np.float32)
    att_src2 = np.asarray(att_src2, np.float32)
    att_dst2 = np.asarray(att_dst2, np.float32)

    # fused weights
    w_asrc1 = np.stack(
        [W1[:, h * ch : (h + 1) * ch] @ att_src1[h] for h in range(nh)], axis=1
    )  # [F, nh]
    w_adst1 = np.stack(
        [W1[:, h * ch : (h + 1) * ch] @ att_dst1[h] for h in range(nh)], axis=1
    )
    w1p = np.concatenate([W1, w_asrc1, w_adst1], axis=1).astype(NP16)  # [F, REC]
    w_asrc2 = W2 @ att_src2[0]
    w_adst2 = W2 @ att_dst2[0]
    w2p = np.concatenate(
        [W2, w_asrc2[:, None], w_adst2[:, None]], axis=1
    ).astype(NP16)  # [D1, 4]

    x = np.asarray(x, np.float32)

    # ---- launch A: per-node R1 ----
    in_maps0 = []
    for c in range(n_cores):
        ids = orders[c]
        real = ids >= 0
        xs = np.where(real[:, None], x[np.maximum(ids, 0)], 0.0)  # [padn, F]
        in_maps0.append(
            {"xt": np.ascontiguousarray(xs.T.astype(NP16)), "w1p": w1p}
        )
    nc0 = _build_a(padn, fdim)
    _split_waits(nc0)
    import time as _time

    t0 = _time.perf_counter()
    res0 = run_bass_kernel_spmd(nc0, in_maps0, list(range(n_cores)))
    t1 = _time.perf_counter()
    if timing is not None:
        timing["a_first_s"] = t1 - t0
        timing["nc0"] = nc0
        timing["in_maps0"] = in_maps0

    # R1 lookup table: [h1(32) | a_src(2) | a_dst(2)], pad row kills e
    r1tab = np.zeros((n_nodes + 1, REC), NP16)
    r1tab[n_nodes, D1 : D1 + NH] = BIG_NEG
    for c in range(n_cores):
        ids = orders[c]
        real = ids >= 0
        r1tab[ids[real]] = res0.results[c]["r1"][:, real].T

    pslots = [
        _padded_slots(slot_src[c], Dt, groups, n_nodes) for c in range(n_cores)
    ]

    # ---- launch B inputs: grouped channel-major slot planes ----
    in_maps1 = []
    b1r = np.broadcast_to(np.asarray(b1, NP16), (TILE, D1)).copy()
    for c in range(n_cores):
        chunks = []
        for gi, (t0g, ng, dg) in enumerate(groups):
            g = r1tab[pslots[c][gi]]  # [L, TILE, REC]
            gt = g.transpose(1, 2, 0)  # [TILE, REC, L]
            hp = gt[:, 0:D1, :]
            asr = gt[:, D1 : D1 + NH, :]
            ids = orders[c][t0g * TILE : (t0g + ng) * TILE].reshape(ng, TILE)
            adv = r1tab[np.maximum(ids, 0), D1 + NH : REC]  # [ng, TILE, NH]
            adv = adv * (ids >= 0)[:, :, None].astype(NP16)
            ade = np.repeat(adv, dg, axis=0)  # [L, TILE, NH]
            ade = ade.transpose(1, 2, 0)  # [TILE, NH, L]
            chunks.append(
                np.concatenate([hp, asr, ade], axis=1).reshape(TILE, 36 * ng * dg)
            )
        in_maps1.append(
            {
                "hsd": np.ascontiguousarray(np.concatenate(chunks, axis=1)),
                "b1r": b1r,
                "w2p": w2p,
            }
        )

    nc1 = _build_l1(Dt, groups, ntiles, padn)
    _split_waits(nc1)
    t2 = _time.perf_counter()
    res1 = run_bass_kernel_spmd(nc1, in_maps1, list(range(n_cores)))
    t3 = _time.perf_counter()
    if timing is not None:
        timing["l1_first_s"] = t3 - t2
        timing["nc1"] = nc1
        timing["in_maps1"] = in_maps1

    # R2 lookup table: [h2(2) | a_src2 | a_dst2]
    r2tab = np.zeros((n_nodes + 1, 4), NP16)
    r2tab[n_nodes, 2] = BIG_NEG
    for c in range(n_cores):
        ids = orders[c]
        real = ids >= 0
        r2tab[ids[real]] = res1.results[c]["r2"][real]

    # ---- launch C inputs: grouped planar slots ----
    in_maps2 = []
    b2r = np.broadcast_to(np.asarray(b2, np.float32), (TILE, 2)).copy()
    for c in range(n_cores):
        chunks = []
        for gi, (t0g, ng, dg) in enumerate(groups):
            g = r2tab[pslots[c][gi]]  # [L, TILE, 4]
            gt = g.transpose(1, 2, 0)  # [TILE, 4, L]
            h2 = gt[:, 0:2, :]
            as2 = gt[:, 2:3, :]
            ids = orders[c][t0g * TILE : (t0g + ng) * TILE].reshape(ng, TILE)
            adv = r2tab[np.maximum(ids, 0), 3] * (ids >= 0).astype(NP16)
            ad2 = np.repeat(adv, dg, axis=0).T[:, None, :]  # [TILE, 1, L]
            chunks.append(
                np.concatenate([h2, as2, ad2], axis=1).reshape(TILE, 4 * ng * dg)
            )
        in_maps2.append(
            {
                "xed": np.ascontiguousarray(np.concatenate(chunks, axis=1)),
                "b2r": b2r,
            }
        )

    nc2 = _build_l2(Dt, groups, ntiles, padn)
    _split_waits(nc2)
    t4 = _time.perf_counter()
    res2 = run_bass_kernel_spmd(nc2, in_maps2, list(range(n_cores)))
    t5 = _time.perf_counter()
    if timing is not None:
        timing["l2_first_s"] = t5 - t4
        timing["nc2"] = nc2
        timing["in_maps2"] = in_maps2

    out = np.zeros((n_nodes, 2), np.float32)
    for c in range(n_cores):
        yc = res2.results[c]["y"]
        ids = orders[c]
        real = ids >= 0
        out[ids[real]] = yc[real]
    return out


def kernel(x, edge_index, W1, att_src1, att_dst1, b1, W2, att_src2, att_dst2, b2):
    return _run_gat(
        np.asarray(x, np.float32),
        np.asarray(edge_index),
        W1,
        att_src1,
        att_dst1,
        b1,
        W2,
        att_src2,
        att_dst2,
        b2,
    )


# revision 22
# speedup vs baseline: 1.0428x; 1.0428x over previous
"""GAT (2-layer, PyG-style) on 8 Trainium2 NeuronCores.

Strategy (edge parallelism per the sharding hint), v3 — three launches:
  - Nodes are split into 8 contiguous ranges (12500/core); each core owns all
    in-edges of its nodes (~412K edges, uniform since the graph is random).
    Per-core nodes are degree-sorted into 128-lane tiles. Tiles are packed
    into groups with a UNIFORM slot depth Dg per group (pad slots reference a
    sentinel row whose a_src = -30000 so e underflows to exactly 0; ~3.5%
    padding thanks to the degree sort).
  - Launch A (node pass): R1 = [h1 | a_src1 | a_dst1] = x @ [W1|W1@As|W1@Ad]
    once per node on the PE (fp16, weights stationary, channel-major out).
  - Host gathers R1[src] per edge slot (72B/edge fp16; 3.5x less HBM than
    raw-x gather, no per-edge matmuls).
  - Launch B (layer-1 edge pass): per group, stream channel-major planes
    [h(32) | a_src(2) | a_dst(2)] fp16; e = exp(lrelu(a_src+a_dst)) (exp on
    ACT); V = e*h with one big packed-fp16 DVE mult (fast path ~0.19ns/elem);
    segment-sum via IN-PLACE fp16 halving-tree adds over the uniform slot
    axis (elementwise fp16 is ~6x faster than tensor_reduce on HW), final
    pair summed to f32. Channel-major finishing: normalize, +b1, ELU,
    R2 = eluT @ [W2|W2@As2|W2@Ad2] via PE transposes.
  - Host gathers R2[src] per edge slot (8B/edge); launch C does layer 2 the
    same way; with only 2 output channels the softmax/log_softmax needs no
    reduces at all (channel-pair tensor_tensor ops).
"""

import sys

sys.path.insert(0, "/opt/trn_rl_repo")

from contextlib import ExitStack

import numpy as np

import concourse.tile as tile
from concourse import bass, mybir
from concourse.bass_utils import run_bass_kernel_spmd
from concourse.masks import make_identity

F32 = mybir.dt.float32
F16 = mybir.dt.float16
NP16 = np.float16

NC = 8
TILE = 128
NH = 2
CH = 16
D1 = NH * CH  # 32
REC = D1 + 2 * NH  # 36
NEG_SLOPE = 0.2
BIG_NEG = -30000.0  # fp16-safe; 0.2*BIG_NEG underflows exp to exactly 0
L_BUDGET = 384  # max (padded) slot columns per group


_ws_seq = [0]


def _split_waits(nc, limit=1):
    """The walrus build in this container rejects instructions carrying more
    than one sem wait ("Too many sync wait commands"). Hoist excess waits
    onto NOP carriers inserted just before the instruction (same engine, same
    program order, so semantics are preserved)."""
    for f in nc.m.functions:
        for blk in f.blocks:
            il = list(blk.instructions)
            out = []
            changed = False
            for inst in il:
                si = inst.sync_info
                waits = list(si.on_wait) if (si and si.on_wait) else []
                if len(waits) > limit:
                    keep = waits[-limit:]
                    for w in waits[:-limit]:
                        _ws_seq[0] += 1
                        nop = mybir.InstNoOp(name=f"WS-{_ws_seq[0]}")
                        nop.engine = inst.engine
                        nop.sync_info = mybir.SyncInfo(on_wait=[w], on_update=[])
                        out.append(nop)
                    si.on_wait = keep
                    changed = True
                out.append(inst)
            if changed:
                blk.instructions = out


# ---------------------------------------------------------------- host prep


def _plan(src, dst, n_nodes, n_cores):
    """Node ranges, degree-sorted tiles, shared D_t schedule, slot src ids."""
    per = n_nodes // n_cores
    ntiles = (per + TILE - 1) // TILE
    padn = ntiles * TILE

    deg = np.bincount(dst, minlength=n_nodes)

    # edges sorted by dst, self-loop (src==dst) first within each segment
    order_e = np.lexsort((src != dst, dst))
    s_src = src[order_e]
    rowptr = np.zeros(n_nodes + 1, dtype=np.int64)
    np.cumsum(deg, out=rowptr[1:])

    orders = []  # per core: global node id per sorted slot lane (-1 = fake)
    Dt_all = np.zeros((n_cores, ntiles), dtype=np.int64)
    for c in range(n_cores):
        d = deg[c * per : (c + 1) * per]
        ids = np.concatenate(
            [c * per + np.arange(per), np.full(padn - per, -1, np.int64)]
        )
        dd = np.concatenate([d, np.zeros(padn - per, np.int64)])
        o = np.argsort(dd, kind="stable")
        orders.append(ids[o])
        Dt_all[c] = dd[o].reshape(ntiles, TILE).max(axis=1)
    Dt = Dt_all.max(axis=0)
    Dt = np.maximum(Dt, 1)  # avoid zero-size tiles
    nblocks = int(Dt.sum())

    # slot src ids per core: [nblocks, TILE] int64, pad = n_nodes
    slot_src = np.full((n_cores, nblocks, TILE), n_nodes, dtype=np.int64)
    for c in range(n_cores):
        ids = orders[c]
        b0 = 0
        for t in range(ntiles):
            D = int(Dt[t])
            nid = ids[t * TILE : (t + 1) * TILE]
            real = nid >= 0
            nid_c = np.where(real, nid, 0)
            degs = np.where(real, deg[nid_c], 0)
            jj = np.arange(D)[:, None]  # [D, TILE]
            valid = jj < degs[None, :]
            eidx = rowptr[nid_c][None, :] + np.minimum(jj, np.maximum(degs - 1, 0))
            vals = s_src[np.clip(eidx, 0, len(s_src) - 1)]
            slot_src[c, b0 : b0 + D] = np.where(valid, vals, n_nodes)
            b0 += D
    return per, ntiles, padn, Dt, nblocks, slot_src, orders


def _groups(Dt):
    """Pack degree-sorted tiles into groups with a UNIFORM padded slot depth.

    Returns [(t0, ng, Dg)]: tiles t0..t0+ng-1, each padded to Dg slots."""
    groups = []
    t0 = 0
    for t in range(len(Dt)):
        if (t - t0 + 1) * int(Dt[t]) > L_BUDGET and t > t0:
            groups.append((t0, t - t0, -(-int(Dt[t - 1]) // 4) * 4))
            t0 = t
    groups.append((t0, len(Dt) - t0, -(-int(Dt[-1]) // 4) * 4))
    return groups


def _padded_slots(slot_src_c, Dt, groups, n_nodes):
    """Per-group padded slot-src tables: [L_g, TILE] with sentinel pads."""
    out = []
    blk = 0
    for t0, ng, dg in groups:
        tab = np.full((ng, dg, TILE), n_nodes, dtype=np.int64)
        for i in range(ng):
            D = int(Dt[t0 + i])
            tab[i, 0:D] = slot_src_c[blk : blk + D]
            blk += D
        out.append(tab.reshape(ng * dg, TILE))
    return out


# ------------------------------------------------------- launch A (node pass)


def _build_a(padn, fdim, repeat=None):
    """R1 = [h1 | a_src1 | a_dst1] = w1p.T @ x, channel-major out [REC, padn]."""
    nc = bass.Bass("TRN2")
    xt = nc.declare_dram_parameter("xt", [fdim, padn], F16, isOutput=False)
    w1p = nc.declare_dram_parameter("w1p", [fdim, REC], F16, isOutput=False)
    r1 = nc.declare_dram_parameter("r1", [REC, padn], F16, isOutput=True)
    nt = padn // TILE

    with ExitStack() as ctx:
        tc = ctx.enter_context(tile.TileContext(nc))
        const = ctx.enter_context(tc.tile_pool(name="const", bufs=1))
        xp = ctx.enter_context(tc.tile_pool(name="xp", bufs=1))
        pp = ctx.enter_context(tc.tile_pool(name="pp", bufs=2, space="PSUM"))
        op = ctx.enter_context(tc.tile_pool(name="op", bufs=1))

        w1t = const.tile([fdim, REC], F16)
        nc.sync.dma_start(out=w1t[:], in_=w1p[:])

        if repeat:
            ctx.enter_context(tc.For_i(0, repeat, 1))
        xtile = xp.tile([fdim, padn], F16, tag="xt")
        nc.sync.dma_start(out=xtile[:], in_=xt[:])
        r1sb = op.tile([REC, padn], F16, tag="r1sb")
        p1 = None
        for t in range(nt):
            q = t % 16
            if q == 0:
                p1 = pp.tile([REC, 16 * TILE], F32, tag="p1")
            nc.tensor.matmul(
                out=p1[:, q * TILE : (q + 1) * TILE],
                lhsT=w1t[:],
                rhs=xtile[:, t * TILE : (t + 1) * TILE],
                start=True,
                stop=True,
            )
            if q == 15 or t == nt - 1:
                nc.vector.tensor_copy(
                    out=r1sb[:, (t - q) * TILE : (t + 1) * TILE],
                    in_=p1[:, 0 : (q + 1) * TILE],
                )
        nc.sync.dma_start(out=r1[:], in_=r1sb[:])
    return nc


# ------------------------------------------------------------- launch B (L1)


def _build_l1(Dt, groups, ntiles, padn, repeat=None):
    """Layer-1 edge pass from host-gathered channel-major slot planes.

    Group stages are issued PAIR-INTERLEAVED: on this HW a DVE instruction
    that waits on the immediately preceding instruction stalls ~2.7us, and
    the in-order queue serializes the stalls; alternating two independent
    groups' instructions hides the dependency latency. Segment-sum = one
    fp16 halving level + one fp16->fp16 tensor_reduce per group."""
    cb = 36 * sum(ng * dg for _, ng, dg in groups)
    lgm = max(ng * dg for _, ng, dg in groups)
    caph = D1 * max(ng * (dg // 2 + dg % 2) for _, ng, dg in groups)
    nc = bass.Bass("TRN2")
    hsd = nc.declare_dram_parameter("hsd", [TILE, cb], F16, isOutput=False)
    b1r = nc.declare_dram_parameter("b1r", [TILE, D1], F16, isOutput=False)
    w2p = nc.declare_dram_parameter("w2p", [D1, 4], F16, isOutput=False)
    r2 = nc.declare_dram_parameter("r2", [padn, 4], F16, isOutput=True)

    offs = []
    off = 0
    for _, ng, dg in groups:
        offs.append(off)
        off += 36 * ng * dg

    with ExitStack() as ctx:
        tc = ctx.enter_context(tile.TileContext(nc))
        const = ctx.enter_context(tc.tile_pool(name="const", bufs=1))
        hspool = ctx.enter_context(tc.tile_pool(name="hs", bufs=2))
        wk = ctx.enter_context(tc.tile_pool(name="wk", bufs=2))
        vpool = ctx.enter_context(tc.tile_pool(name="vp", bufs=2))
        lvl = ctx.enter_context(tc.tile_pool(name="lvl", bufs=2))
        ppool = ctx.enter_context(tc.tile_pool(name="pp", bufs=2, space="PSUM"))
        rpool = ctx.enter_context(tc.tile_pool(name="rp", bufs=2, space="PSUM"))
        outp = ctx.enter_context(tc.tile_pool(name="op", bufs=1))

        b1t = const.tile([TILE, D1], F16)
        nc.sync.dma_start(out=b1t[:], in_=b1r[:])
        w2t = const.tile([D1, 4], F16)
        nc.sync.dma_start(out=w2t[:], in_=w2p[:])
        ident = const.tile([TILE, TILE], F16)
        make_identity(nc, ident[:])

        if repeat:
            ctx.enter_context(tc.For_i(0, repeat, 1))
        o1a16 = outp.tile([TILE, D1, ntiles], F16, tag="o1a16")
        sa16 = outp.tile([TILE, NH, ntiles], F16, tag="sa16")

        st = {}
        pairs = [groups[i : i + 2] for i in range(0, len(groups), 2)]
        gidx = 0
        for pair in pairs:
            ids = list(range(gidx, gidx + len(pair)))
            gidx += len(pair)
            for gi, (t0, ng, dg) in zip(ids, pair):
                L = ng * dg
                hst = hspool.tile([TILE, 36, lgm], F16, tag="hst")
                nc.sync.dma_start(
                    out=hst[:, :, 0:L],
                    in_=hsd[:, offs[gi] : offs[gi] + 36 * L].rearrange(
                        "p (c j) -> p c j", c=36
                    ),
                )
                st[gi] = {"hst": hst}
            for gi, (t0, ng, dg) in zip(ids, pair):
                L = ng * dg
                lg = wk.tile([TILE, NH, lgm], F16, tag="lg")
                nc.vector.tensor_tensor(
                    out=lg[:, :, 0:L],
                    in0=st[gi]["hst"][:, D1 : D1 + NH, 0:L],
                    in1=st[gi]["hst"][:, D1 + NH : REC, 0:L],
                    op=mybir.AluOpType.add,
                )
                st[gi]["lg"] = lg
            for gi, (t0, ng, dg) in zip(ids, pair):
                L = ng * dg
                ls = wk.tile([TILE, NH, lgm], F16, tag="ls")
                nc.vector.tensor_scalar_mul(
                    out=ls[:, :, 0:L], in0=st[gi]["lg"][:, :, 0:L],
                    scalar1=NEG_SLOPE,
                )
                st[gi]["ls"] = ls
            for gi, (t0, ng, dg) in zip(ids, pair):
                L = ng * dg
                nc.vector.tensor_tensor(
                    out=st[gi]["lg"][:, :, 0:L],
                    in0=st[gi]["lg"][:, :, 0:L],
                    in1=st[gi]["ls"][:, :, 0:L],
                    op=mybir.AluOpType.max,
                )
            for gi, (t0, ng, dg) in zip(ids, pair):
                L = ng * dg
                et = wk.tile([TILE, NH, lgm], F16, tag="et")
                nc.scalar.activation(
                    out=et[:, :, 0:L], in_=st[gi]["lg"][:, :, 0:L],
                    func=mybir.ActivationFunctionType.Exp,
                )
                st[gi]["et"] = et
            for gi, (t0, ng, dg) in zip(ids, pair):
                L = ng * dg
                V = vpool.tile([TILE, NH, CH, lgm], F16, tag="V")
                nc.vector.tensor_tensor(
                    out=V[:, :, :, 0:L],
                    in0=st[gi]["hst"][:, 0:D1, 0:L].rearrange(
                        "p (h c) j -> p h c j", h=NH
                    ),
                    in1=st[gi]["et"][:, :, 0:L]
                    .unsqueeze(2)
                    .to_broadcast([TILE, NH, CH, L]),
                    op=mybir.AluOpType.mult,
                )
                st[gi]["V"] = V
            for gi, (t0, ng, dg) in zip(ids, pair):
                # halving level 1: Vh[.., j] = V[.., j] + V[.., k+j]
                L = ng * dg
                k = dg // 2
                flat = lvl.tile([TILE, caph], F16, tag="lv")
                Vh = flat[:, 0 : D1 * ng * k].rearrange(
                    "p (c t j) -> p c t j", c=D1, t=ng
                )
                V5 = st[gi]["V"][:, :, :, 0:L].rearrange(
                    "p h c (t j) -> p (h c) t j", j=dg
                )
                nc.vector.tensor_tensor(
                    out=Vh[:, :, :, 0:k],
                    in0=V5[:, :, :, 0:k],
                    in1=V5[:, :, :, k : 2 * k],
                    op=mybir.AluOpType.add,
                )
                st[gi]["Vh"] = Vh
            for gi, (t0, ng, dg) in zip(ids, pair):
                # halving level 2
                k = dg // 2
                k2 = k // 2
                flat2 = lvl.tile([TILE, caph // 2 + D1], F16, tag="lv2")
                Vh2 = flat2[:, 0 : D1 * ng * k2].rearrange(
                    "p (c t j) -> p c t j", c=D1, t=ng
                )
                nc.vector.tensor_tensor(
                    out=Vh2[:, :, :, 0:k2],
                    in0=st[gi]["Vh"][:, :, :, 0:k2],
                    in1=st[gi]["Vh"][:, :, :, k2 : 2 * k2],
                    op=mybir.AluOpType.add,
                )
                st[gi]["Vh"] = Vh2
            with nc.allow_low_precision(reason="fp16 segment sums, 2e-2 gate"):
                for gi, (t0, ng, dg) in zip(ids, pair):
                    nc.vector.tensor_reduce(
                        out=o1a16[:, :, t0 : t0 + ng],
                        in_=st[gi]["Vh"],
                        axis=mybir.AxisListType.X,
                        op=mybir.AluOpType.add,
                    )
                for gi, (t0, ng, dg) in zip(ids, pair):
                    L = ng * dg
                    E5 = st[gi]["et"][:, :, 0:L].rearrange(
                        "p h (t j) -> p h t j", j=dg
                    )
                    nc.vector.tensor_reduce(
                        out=sa16[:, :, t0 : t0 + ng],
                        in_=E5,
                        axis=mybir.AxisListType.X,
                        op=mybir.AluOpType.add,
                    )
            st.clear()

        # ---- finishing: two tile-range halves, instruction-interleaved ----
        inv = outp.tile([TILE, NH, ntiles], F32, tag="inv")
        invh = outp.tile([TILE, NH, ntiles], F16, tag="invh")
        o1f = outp.tile([TILE, NH, CH, ntiles], F16, tag="o1f")
        e1 = outp.tile([TILE, NH, CH, ntiles], F16, tag="e1")

        def fin_chain(ts, te):
            w = te - ts
            yield lambda: nc.vector.tensor_scalar_add(
                out=inv[:, :, ts:te], in0=sa16[:, :, ts:te], scalar1=1e-16
            )
            yield lambda: nc.vector.reciprocal(
                out=inv[:, :, ts:te], in_=inv[:, :, ts:te]
            )
            yield lambda: nc.vector.tensor_copy(
                out=invh[:, :, ts:te], in_=inv[:, :, ts:te]
            )
            yield lambda: nc.vector.tensor_tensor(
                out=o1f[:, :, :, ts:te],
                in0=o1a16[:, :, ts:te].rearrange("p (h c) t -> p h c t", h=NH),
                in1=invh[:, :, ts:te]
                .unsqueeze(2)
                .to_broadcast([TILE, NH, CH, w]),
                op=mybir.AluOpType.mult,
            )
            yield lambda: nc.vector.tensor_tensor(
                out=o1f[:, :, :, ts:te],
                in0=o1f[:, :, :, ts:te],
                in1=b1t[:]
                .rearrange("p (h c) -> p h c", h=NH)
                .unsqueeze(-1)
                .to_broadcast([TILE, NH, CH, w]),
                op=mybir.AluOpType.add,
            )
            yield lambda: nc.vector.tensor_scalar_min(
                out=e1[:, :, :, ts:te], in0=o1f[:, :, :, ts:te], scalar1=0.0
            )
            yield lambda: nc.scalar.activation(
                out=e1[:, :, :, ts:te],
                in_=e1[:, :, :, ts:te],
                func=mybir.ActivationFunctionType.Exp,
            )
            yield lambda: nc.vector.tensor_scalar_add(
                out=e1[:, :, :, ts:te], in0=e1[:, :, :, ts:te], scalar1=-1.0
            )
            yield lambda: nc.vector.tensor_scalar_max(
                out=o1f[:, :, :, ts:te], in0=o1f[:, :, :, ts:te], scalar1=0.0
            )
            yield lambda: nc.vector.tensor_tensor(
                out=o1f[:, :, :, ts:te],
                in0=o1f[:, :, :, ts:te],
                in1=e1[:, :, :, ts:te],
                op=mybir.AluOpType.add,
            )

        for s in fin_chain(0, ntiles):
            s()

        # R2 = [h2 | a_src2 | a_dst2] = elu_out @ w2p via PE transposes
        o1tsb = outp.tile([D1, padn], F16, tag="o1t")
        pt = None
        for t in range(ntiles):
            q = t % 4
            if q == 0:
                pt = ppool.tile([D1, 4 * TILE], F16, tag="pt")
            nc.tensor.transpose(
                out=pt[:, q * TILE : (q + 1) * TILE],
                in_=o1f[:, :, :, t].rearrange("p h c -> p (h c)"),
                identity=ident[:],
            )
            if q == 3 or t == ntiles - 1:
                nc.vector.tensor_copy(
                    out=o1tsb[:, (t - q) * TILE : (t + 1) * TILE],
                    in_=pt[:, 0 : (q + 1) * TILE],
                )
        r2all = outp.tile([TILE, ntiles, 4], F16, tag="r2all")
        r2p = None
        for t in range(ntiles):
            q = t % 32
            if q == 0:
                r2p = rpool.tile([TILE, 32 * 4], F32, tag="r2p")
            nc.tensor.matmul(
                out=r2p[:, q * 4 : (q + 1) * 4],
                lhsT=o1tsb[:, t * TILE : (t + 1) * TILE],
                rhs=w2t[:],
                start=True,
                stop=True,
            )
            if q == 31 or t == ntiles - 1:
                nc.vector.tensor_copy(
                    out=r2all[:, t - q : t + 1, :],
                    in_=r2p[:, 0 : (q + 1) * 4].rearrange("p (t c) -> p t c", c=4),
                )
        nc.sync.dma_start(
            out=r2[:].rearrange("(t n) c -> n t c", n=TILE), in_=r2all[:]
        )
    return nc


# ------------------------------------------------------------- launch C (L2)


def _build_l2(Dt, groups, ntiles, padn, repeat=None):
    """Layer 2 (1 head, 2 ch) from grouped planar [h2(2) | a_src2 | a_dst2]
    slots, bias and log_softmax (channel-pair ops, no reduces in finishing).
    Group stages pair-interleaved as in _build_l1."""
    cb = 4 * sum(ng * dg for _, ng, dg in groups)
    lgm = max(ng * dg for _, ng, dg in groups)
    nc = bass.Bass("TRN2")
    xed = nc.declare_dram_parameter("xed", [TILE, cb], F16, isOutput=False)
    b2r = nc.declare_dram_parameter("b2r", [TILE, 2], F32, isOutput=False)
    y = nc.declare_dram_parameter("y", [padn, 2], F32, isOutput=True)

    offs = []
    off = 0
    for _, ng, dg in groups:
        offs.append(off)
        off += 4 * ng * dg

    with ExitStack() as ctx:
        tc = ctx.enter_context(tile.TileContext(nc))
        const = ctx.enter_context(tc.tile_pool(name="const", bufs=1))
        xp = ctx.enter_context(tc.tile_pool(name="xp", bufs=2))
        wk = ctx.enter_context(tc.tile_pool(name="wk", bufs=2))
        outp = ctx.enter_context(tc.tile_pool(name="op", bufs=1))

        b2t = const.tile([TILE, 2], F32)
        nc.sync.dma_start(out=b2t[:], in_=b2r[:])

        if repeat:
            ctx.enter_context(tc.For_i(0, repeat, 1))
        acc2 = outp.tile([TILE, 2, ntiles], F16, tag="acc2")
        s2h = outp.tile([TILE, ntiles], F16, tag="s2h")

        st = {}
        pairs = [groups[i : i + 2] for i in range(0, len(groups), 2)]
        gidx = 0
        for pair in pairs:
            ids = list(range(gidx, gidx + len(pair)))
            gidx += len(pair)
            for gi, (t0, ng, dg) in zip(ids, pair):
                L = ng * dg
                xe = xp.tile([TILE, 4, lgm], F16, tag="xe")
                nc.sync.dma_start(
                    out=xe[:, :, 0:L],
                    in_=xed[:, offs[gi] : offs[gi] + 4 * L].rearrange(
                        "p (c j) -> p c j", c=4
                    ),
                )
                st[gi] = {"xe": xe}
            for gi, (t0, ng, dg) in zip(ids, pair):
                L = ng * dg
                lg = wk.tile([TILE, lgm], F16, tag="lg")
                nc.vector.tensor_tensor(
                    out=lg[:, 0:L], in0=st[gi]["xe"][:, 2, 0:L],
                    in1=st[gi]["xe"][:, 3, 0:L], op=mybir.AluOpType.add,
                )
                st[gi]["lg"] = lg
            for gi, (t0, ng, dg) in zip(ids, pair):
                L = ng * dg
                ls = wk.tile([TILE, lgm], F16, tag="ls")
                nc.vector.tensor_scalar_mul(
                    out=ls[:, 0:L], in0=st[gi]["lg"][:, 0:L], scalar1=NEG_SLOPE
                )
                st[gi]["ls"] = ls
            for gi, (t0, ng, dg) in zip(ids, pair):
                L = ng * dg
                nc.vector.tensor_tensor(
                    out=st[gi]["lg"][:, 0:L], in0=st[gi]["lg"][:, 0:L],
                    in1=st[gi]["ls"][:, 0:L], op=mybir.AluOpType.max,
                )
            for gi, (t0, ng, dg) in zip(ids, pair):
                L = ng * dg
                et = wk.tile([TILE, lgm], F16, tag="et")
                nc.scalar.activation(
                    out=et[:, 0:L], in_=st[gi]["lg"][:, 0:L],
                    func=mybir.ActivationFunctionType.Exp,
                )
                st[gi]["et"] = et
            for gi, (t0, ng, dg) in zip(ids, pair):
                L = ng * dg
                V = wk.tile([TILE, 2, lgm], F16, tag="V")
                nc.vector.tensor_tensor(
                    out=V[:, :, 0:L],
                    in0=st[gi]["xe"][:, 0:2, 0:L],
                    in1=st[gi]["et"][:, 0:L]
                    .unsqueeze(1)
                    .to_broadcast([TILE, 2, L]),
                    op=mybir.AluOpType.mult,
                )
                st[gi]["V"] = V
            with nc.allow_low_precision(reason="fp16 segment sums, 2e-2 gate"):
                for gi, (t0, ng, dg) in zip(ids, pair):
                    L = ng * dg
                    V5 = st[gi]["V"][:, :, 0:L].rearrange(
                        "p c (t j) -> p c t j", j=dg
                    )
                    nc.vector.tensor_reduce(
                        out=acc2[:, :, t0 : t0 + ng],
                        in_=V5,
                        axis=mybir.AxisListType.X,
                        op=mybir.AluOpType.add,
                    )
                for gi, (t0, ng, dg) in zip(ids, pair):
                    L = ng * dg
                    E5 = st[gi]["et"][:, 0:L].rearrange(
                        "p (t j) -> p t j", j=dg
                    )
                    nc.vector.tensor_reduce(
                        out=s2h[:, t0 : t0 + ng],
                        in_=E5,
                        axis=mybir.AxisListType.X,
                        op=mybir.AluOpType.add,
                    )
            st.clear()

        # ---- finishing: two tile-range halves, instruction-interleaved ----
        inv = outp.tile([TILE, ntiles], F32, tag="inv")
        z = outp.tile([TILE, 2, ntiles], F32, tag="z")
        m = outp.tile([TILE, ntiles], F32, tag="m")
        ez = outp.tile([TILE, 2, ntiles], F32, tag="ez")
        ss = outp.tile([TILE, ntiles], F32, tag="ss")
        yt = outp.tile([TILE, 2, ntiles], F32, tag="yt")
        yt2 = outp.tile([TILE, ntiles, 2], F32, tag="yt2")

        def fin_chain(ts, te):
            w = te - ts
            yield lambda: nc.vector.tensor_scalar_add(
                out=inv[:, ts:te], in0=s2h[:, ts:te], scalar1=1e-16
            )
            yield lambda: nc.vector.reciprocal(
                out=inv[:, ts:te], in_=inv[:, ts:te]
            )
            yield lambda: nc.vector.tensor_tensor(
                out=z[:, :, ts:te],
                in0=acc2[:, :, ts:te],
                in1=inv[:, ts:te].unsqueeze(1).to_broadcast([TILE, 2, w]),
                op=mybir.AluOpType.mult,
            )
            yield lambda: nc.vector.tensor_tensor(
                out=z[:, :, ts:te],
                in0=z[:, :, ts:te],
                in1=b2t[:].unsqueeze(-1).to_broadcast([TILE, 2, w]),
                op=mybir.AluOpType.add,
            )
            yield lambda: nc.vector.tensor_tensor(
                out=m[:, ts:te], in0=z[:, 0, ts:te], in1=z[:, 1, ts:te],
                op=mybir.AluOpType.max,
            )
            yield lambda: nc.vector.tensor_tensor(
                out=z[:, :, ts:te],
                in0=z[:, :, ts:te],
                in1=m[:, ts:te].unsqueeze(1).to_broadcast([TILE, 2, w]),
                op=mybir.AluOpType.subtract,
            )
            yield lambda: nc.scalar.activation(
                out=ez[:, :, ts:te], in_=z[:, :, ts:te],
                func=mybir.ActivationFunctionType.Exp,
            )
            yield lambda: nc.vector.tensor_tensor(
                out=ss[:, ts:te], in0=ez[:, 0, ts:te], in1=ez[:, 1, ts:te],
                op=mybir.AluOpType.add,
            )
            yield lambda: nc.scalar.activation(
                out=ss[:, ts:te], in_=ss[:, ts:te],
                func=mybir.ActivationFunctionType.Ln,
            )
            yield lambda: nc.vector.tensor_tensor(
                out=yt[:, :, ts:te],
                in0=z[:, :, ts:te],
                in1=ss[:, ts:te].unsqueeze(1).to_broadcast([TILE, 2, w]),
                op=mybir.AluOpType.subtract,
            )
            yield lambda: nc.vector.tensor_copy(
                out=yt2[:, ts:te, :],
                in_=yt[:, :, ts:te].rearrange("p c t -> p t c"),
            )

        for s in fin_chain(0, ntiles):
            s()
        nc.sync.dma_start(
            out=y[:].rearrange("(t n) c -> n t c", n=TILE), in_=yt2[:]
        )
    return nc


# ------------------------------------------------------------------- driver


def _run_gat(x, edge_index, W1, att_src1, att_dst1, b1, W2, att_src2, att_dst2, b2,
             n_cores=NC, timing=None):
    n_nodes, fdim = x.shape
    nh, ch = att_src1.shape

    src = np.concatenate([np.asarray(edge_index[0]), np.arange(n_nodes)]).astype(
        np.int64
    )
    dst = np.concatenate([np.asarray(edge_index[1]), np.arange(n_nodes)]).astype(
        np.int64
    )

    per, ntiles, padn, Dt, nblocks, slot_src, orders = _plan(
        src, dst, n_nodes, n_cores
    )
    groups = _groups(Dt)

    W1 = np.asarray(W1, np.float32)
    att_src1 = np.asarray(att_src1, np.float32)
    att_dst1 = np.asarray(att_dst1, np.float32)
    W2 = np.asarray(W2, np.float32)
    att_src2 = np.asarray(att_src2, np.float32)
    att_dst2 = np.asarray(att_dst2, np.float32)

    # fused weights
    w_asrc1 = np.stack(
        [W1[:, h * ch : (h + 1) * ch] @ att_src1[h] for h in range(nh)], axis=1
    )  # [F, nh]
    w_adst1 = np.stack(
        [W1[:, h * ch : (h + 1) * ch] @ att_dst1[h] for h in range(nh)], axis=1
    )
    w1p = np.concatenate([W1, w_asrc1, w_adst1], axis=1).astype(NP16)  # [F, REC]
    w_asrc2 = W2 @ att_src2[0]
    w_adst2 = W2 @ att_dst2[0]
    w2p = np.concatenate(
        [W2, w_asrc2[:, None], w_adst2[:, None]], axis=1
    ).astype(NP16)  # [D1, 4]

    x = np.asarray(x, np.float32)

    # ---- launch A: per-node R1 ----
    in_maps0 = []
    for c in range(n_cores):
        ids = orders[c]
        real = ids >= 0
        xs = np.where(real[:, None], x[np.maximum(ids, 0)], 0.0)  # [padn, F]
        in_maps0.append(
            {"xt": np.ascontiguousarray(xs.T.astype(NP16)), "w1p": w1p}
        )
    nc0 = _build_a(padn, fdim)
    _split_waits(nc0)
    import time as _time

    t0 = _time.perf_counter()
    res0 = run_bass_kernel_spmd(nc0, in_maps0, list(range(n_cores)))
    t1 = _time.perf_counter()
    if timing is not None:
        timing["a_first_s"] = t1 - t0
        timing["nc0"] = nc0
        timing["in_maps0"] = in_maps0

    # R1 lookup table: [h1(32) | a_src(2) | a_dst(2)], pad row kills e
    r1tab = np.zeros((n_nodes + 1, REC), NP16)
    r1tab[n_nodes, D1 : D1 + NH] = BIG_NEG
    for c in range(n_cores):
        ids = orders[c]
        real = ids >= 0
        r1tab[ids[real]] = res0.results[c]["r1"][:, real].T

    pslots = [
        _padded_slots(slot_src[c], Dt, groups, n_nodes) for c in range(n_cores)
    ]

    # ---- launch B inputs: grouped channel-major slot planes ----
    in_maps1 = []
    b1r = np.broadcast_to(np.asarray(b1, NP16), (TILE, D1)).copy()
    for c in range(n_cores):
        chunks = []
        for gi, (t0g, ng, dg) in enumerate(groups):
            g = r1tab[pslots[c][gi]]  # [L, TILE, REC]
            gt = g.transpose(1, 2, 0)  # [TILE, REC, L]
            hp = gt[:, 0:D1, :]
            asr = gt[:, D1 : D1 + NH, :]
            ids = orders[c][t0g * TILE : (t0g + ng) * TILE].reshape(ng, TILE)
            adv = r1tab[np.maximum(ids, 0), D1 + NH : REC]  # [ng, TILE, NH]
            adv = adv * (ids >= 0)[:, :, None].astype(NP16)
            ade = np.repeat(adv, dg, axis=0)  # [L, TILE, NH]
            ade = ade.transpose(1, 2, 0)  # [TILE, NH, L]
            chunks.append(
                np.concatenate([hp, asr, ade], axis=1).reshape(TILE, 36 * ng * dg)
            )
        in_maps1.append(
            {
                "hsd": np.ascontiguousarray(np.concatenate(chunks, axis=1)),
                "b1r": b1r,
                "w2p": w2p,
            }
        )

    nc1 = _build_l1(Dt, groups, ntiles, padn)
    _split_waits(nc1)
    t2 = _time.perf_counter()
    res1 = run_bass_kernel_spmd(nc1, in_maps1, list(range(n_cores)))
    t3 = _time.perf_counter()
    if timing is not None:
        timing["l1_first_s"] = t3 - t2
        timing["nc1"] = nc1
        timing["in_maps1"] = in_maps1

    # R2 lookup table: [h2(2) | a_src2 | a_dst2]
    r2tab = np.zeros((n_nodes + 1, 4), NP16)
    r2tab[n_nodes, 2] = BIG_NEG
    for c in range(n_cores):
        ids = orders[c]
        real = ids >= 0
        r2tab[ids[real]] = res1.results[c]["r2"][real]

    # ---- launch C inputs: grouped planar slots ----
    in_maps2 = []
    b2r = np.broadcast_to(np.asarray(b2, np.float32), (TILE, 2)).copy()
    for c in range(n_cores):
        chunks = []
        for gi, (t0g, ng, dg) in enumerate(groups):
            g = r2tab[pslots[c][gi]]  # [L, TILE, 4]
            gt = g.transpose(1, 2, 0)  # [TILE, 4, L]
            h2 = gt[:, 0:2, :]
            as2 = gt[:, 2:3, :]
            ids = orders[c][t0g * TILE : (t0g + ng) * TILE].reshape(ng, TILE)
            adv = r2tab[np.maximum(ids, 0), 3] * (ids >= 0).astype(NP16)
            ad2 = np.repeat(adv, dg, axis=0).T[:, None, :]  # [TILE, 1, L]
            chunks.append(
                np.concatenate([h2, as2, ad2], axis=1).reshape(TILE, 4 * ng * dg)
            )
        in_maps2.append(
            {
                "xed": np.ascontiguousarray(np.concatenate(chunks, axis=1)),
                "b2r": b2r,
            }
        )

    nc2 = _build_l2(Dt, groups, ntiles, padn)
    _split_waits(nc2)
    t4 = _time.perf_counter()
    res2 = run_bass_kernel_spmd(nc2, in_maps2, list(range(n_cores)))
    t5 = _time.perf_counter()
    if timing is not None:
        timing["l2_first_s"] = t5 - t4
        timing["nc2"] = nc2
        timing["in_maps2"] = in_maps2

    out = np.zeros((n_nodes, 2), np.float32)
    for c in range(n_cores):
        yc = res2.results[c]["y"]
        ids = orders[c]
        real = ids >= 0
        out[ids[real]] = yc[real]
    return out


def kernel(x, edge_index, W1, att_src1, att_dst1, b1, W2, att_src2, att_dst2, b2):
    return _run_gat(
        np.asarray(x, np.float32),
        np.asarray(edge_index),
        W1,
        att_src1,
        att_dst1,
        b1,
        W2,
        att_src2,
        att_dst2,
        b2,
    )


# revision 23
# speedup vs baseline: 1.0659x; 1.0221x over previous
"""GAT (2-layer, PyG-style) on 8 Trainium2 NeuronCores.

Strategy (edge parallelism per the sharding hint), v3 — three launches:
  - Nodes are split into 8 contiguous ranges (12500/core); each core owns all
    in-edges of its nodes (~412K edges, uniform since the graph is random).
    Per-core nodes are degree-sorted into 128-lane tiles. Tiles are packed
    into groups with a UNIFORM slot depth Dg per group (pad slots reference a
    sentinel row whose a_src = -30000 so e underflows to exactly 0; ~3.5%
    padding thanks to the degree sort).
  - Launch A (node pass): R1 = [h1 | a_src1 | a_dst1] = x @ [W1|W1@As|W1@Ad]
    once per node on the PE (fp16, weights stationary, channel-major out).
  - Host gathers R1[src] per edge slot (72B/edge fp16; 3.5x less HBM than
    raw-x gather, no per-edge matmuls).
  - Launch B (layer-1 edge pass): per group, stream channel-major planes
    [h(32) | a_src(2) | a_dst(2)] fp16; e = exp(lrelu(a_src+a_dst)) (exp on
    ACT); V = e*h with one big packed-fp16 DVE mult (fast path ~0.19ns/elem);
    segment-sum via IN-PLACE fp16 halving-tree adds over the uniform slot
    axis (elementwise fp16 is ~6x faster than tensor_reduce on HW), final
    pair summed to f32. Channel-major finishing: normalize, +b1, ELU,
    R2 = eluT @ [W2|W2@As2|W2@Ad2] via PE transposes.
  - Host gathers R2[src] per edge slot (8B/edge); launch C does layer 2 the
    same way; with only 2 output channels the softmax/log_softmax needs no
    reduces at all (channel-pair tensor_tensor ops).
"""

import sys

sys.path.insert(0, "/opt/trn_rl_repo")

from contextlib import ExitStack

import numpy as np

import concourse.tile as tile
from concourse import bass, mybir
from concourse.bass_utils import run_bass_kernel_spmd
from concourse.masks import make_identity

F32 = mybir.dt.float32
F16 = mybir.dt.float16
NP16 = np.float16

NC = 8
TILE = 128
NH = 2
CH = 16
D1 = NH * CH  # 32
REC = D1 + 2 * NH  # 36
NEG_SLOPE = 0.2
BIG_NEG = -30000.0  # fp16-safe; 0.2*BIG_NEG underflows exp to exactly 0
L_BUDGET = 384  # max (padded) slot columns per group (L1)
L2_BUDGET = 1024  # L2 tiles are tiny; coarser groups = fewer wait-carrying instrs


_ws_seq = [0]


def _split_waits(nc, limit=1):
    """The walrus build in this container rejects instructions carrying more
    than one sem wait ("Too many sync wait commands"). Hoist excess waits
    onto NOP carriers inserted just before the instruction (same engine, same
    program order, so semantics are preserved)."""
    for f in nc.m.functions:
        for blk in f.blocks:
            il = list(blk.instructions)
            out = []
            changed = False
            for inst in il:
                si = inst.sync_info
                waits = list(si.on_wait) if (si and si.on_wait) else []
                if len(waits) > limit:
                    keep = waits[-limit:]
                    for w in waits[:-limit]:
                        _ws_seq[0] += 1
                        nop = mybir.InstNoOp(name=f"WS-{_ws_seq[0]}")
                        nop.engine = inst.engine
                        nop.sync_info = mybir.SyncInfo(on_wait=[w], on_update=[])
                        out.append(nop)
                    si.on_wait = keep
                    changed = True
                out.append(inst)
            if changed:
                blk.instructions = out


# ---------------------------------------------------------------- host prep


def _plan(src, dst, n_nodes, n_cores):
    """Node ranges, degree-sorted tiles, shared D_t schedule, slot src ids."""
    per = n_nodes // n_cores
    ntiles = (per + TILE - 1) // TILE
    padn = ntiles * TILE

    deg = np.bincount(dst, minlength=n_nodes)

    # edges sorted by dst, self-loop (src==dst) first within each segment
    order_e = np.lexsort((src != dst, dst))
    s_src = src[order_e]
    rowptr = np.zeros(n_nodes + 1, dtype=np.int64)
    np.cumsum(deg, out=rowptr[1:])

    orders = []  # per core: global node id per sorted slot lane (-1 = fake)
    Dt_all = np.zeros((n_cores, ntiles), dtype=np.int64)
    for c in range(n_cores):
        d = deg[c * per : (c + 1) * per]
        ids = np.concatenate(
            [c * per + np.arange(per), np.full(padn - per, -1, np.int64)]
        )
        dd = np.concatenate([d, np.zeros(padn - per, np.int64)])
        o = np.argsort(dd, kind="stable")
        orders.append(ids[o])
        Dt_all[c] = dd[o].reshape(ntiles, TILE).max(axis=1)
    Dt = Dt_all.max(axis=0)
    Dt = np.maximum(Dt, 1)  # avoid zero-size tiles
    nblocks = int(Dt.sum())

    # slot src ids per core: [nblocks, TILE] int64, pad = n_nodes
    slot_src = np.full((n_cores, nblocks, TILE), n_nodes, dtype=np.int64)
    for c in range(n_cores):
        ids = orders[c]
        b0 = 0
        for t in range(ntiles):
            D = int(Dt[t])
            nid = ids[t * TILE : (t + 1) * TILE]
            real = nid >= 0
            nid_c = np.where(real, nid, 0)
            degs = np.where(real, deg[nid_c], 0)
            jj = np.arange(D)[:, None]  # [D, TILE]
            valid = jj < degs[None, :]
            eidx = rowptr[nid_c][None, :] + np.minimum(jj, np.maximum(degs - 1, 0))
            vals = s_src[np.clip(eidx, 0, len(s_src) - 1)]
            slot_src[c, b0 : b0 + D] = np.where(valid, vals, n_nodes)
            b0 += D
    return per, ntiles, padn, Dt, nblocks, slot_src, orders


def _groups(Dt, budget=L_BUDGET):
    """Pack degree-sorted tiles into groups with a UNIFORM padded slot depth.

    Returns [(t0, ng, Dg)]: tiles t0..t0+ng-1, each padded to Dg slots."""
    groups = []
    t0 = 0
    for t in range(len(Dt)):
        if (t - t0 + 1) * int(Dt[t]) > budget and t > t0:
            groups.append((t0, t - t0, -(-int(Dt[t - 1]) // 4) * 4))
            t0 = t
    groups.append((t0, len(Dt) - t0, -(-int(Dt[-1]) // 4) * 4))
    return groups


def _padded_slots(slot_src_c, Dt, groups, n_nodes):
    """Per-group padded slot-src tables: [L_g, TILE] with sentinel pads."""
    out = []
    blk = 0
    for t0, ng, dg in groups:
        tab = np.full((ng, dg, TILE), n_nodes, dtype=np.int64)
        for i in range(ng):
            D = int(Dt[t0 + i])
            tab[i, 0:D] = slot_src_c[blk : blk + D]
            blk += D
        out.append(tab.reshape(ng * dg, TILE))
    return out


# ------------------------------------------------------- launch A (node pass)


def _build_a(padn, fdim, repeat=None):
    """R1 = [h1 | a_src1 | a_dst1] = w1p.T @ x, channel-major out [REC, padn]."""
    nc = bass.Bass("TRN2")
    xt = nc.declare_dram_parameter("xt", [fdim, padn], F16, isOutput=False)
    w1p = nc.declare_dram_parameter("w1p", [fdim, REC], F16, isOutput=False)
    r1 = nc.declare_dram_parameter("r1", [REC, padn], F16, isOutput=True)
    nt = padn // TILE

    with ExitStack() as ctx:
        tc = ctx.enter_context(tile.TileContext(nc))
        const = ctx.enter_context(tc.tile_pool(name="const", bufs=1))
        xp = ctx.enter_context(tc.tile_pool(name="xp", bufs=1))
        pp = ctx.enter_context(tc.tile_pool(name="pp", bufs=2, space="PSUM"))
        op = ctx.enter_context(tc.tile_pool(name="op", bufs=1))

        w1t = const.tile([fdim, REC], F16)
        nc.sync.dma_start(out=w1t[:], in_=w1p[:])

        if repeat:
            ctx.enter_context(tc.For_i(0, repeat, 1))
        xtile = xp.tile([fdim, padn], F16, tag="xt")
        nc.sync.dma_start(out=xtile[:], in_=xt[:])
        r1sb = op.tile([REC, padn], F16, tag="r1sb")
        p1 = None
        for t in range(nt):
            q = t % 16
            if q == 0:
                p1 = pp.tile([REC, 16 * TILE], F32, tag="p1")
            nc.tensor.matmul(
                out=p1[:, q * TILE : (q + 1) * TILE],
                lhsT=w1t[:],
                rhs=xtile[:, t * TILE : (t + 1) * TILE],
                start=True,
                stop=True,
            )
            if q == 15 or t == nt - 1:
                nc.vector.tensor_copy(
                    out=r1sb[:, (t - q) * TILE : (t + 1) * TILE],
                    in_=p1[:, 0 : (q + 1) * TILE],
                )
        nc.sync.dma_start(out=r1[:], in_=r1sb[:])
    return nc


# ------------------------------------------------------------- launch B (L1)


def _build_l1(Dt, groups, ntiles, padn, repeat=None):
    """Layer-1 edge pass from host-gathered channel-major slot planes.

    Group stages are issued PAIR-INTERLEAVED: on this HW a DVE instruction
    that waits on the immediately preceding instruction stalls ~2.7us, and
    the in-order queue serializes the stalls; alternating two independent
    groups' instructions hides the dependency latency. Segment-sum = one
    fp16 halving level + one fp16->fp16 tensor_reduce per group."""
    cb = 36 * sum(ng * dg for _, ng, dg in groups)
    lgm = max(ng * dg for _, ng, dg in groups)
    caph = D1 * max(ng * (dg // 2 + dg % 2) for _, ng, dg in groups)
    nc = bass.Bass("TRN2")
    hsd = nc.declare_dram_parameter("hsd", [TILE, cb], F16, isOutput=False)
    b1r = nc.declare_dram_parameter("b1r", [TILE, D1], F16, isOutput=False)
    w2p = nc.declare_dram_parameter("w2p", [D1, 4], F16, isOutput=False)
    r2 = nc.declare_dram_parameter("r2", [padn, 4], F16, isOutput=True)

    offs = []
    off = 0
    for _, ng, dg in groups:
        offs.append(off)
        off += 36 * ng * dg

    with ExitStack() as ctx:
        tc = ctx.enter_context(tile.TileContext(nc))
        const = ctx.enter_context(tc.tile_pool(name="const", bufs=1))
        hspool = ctx.enter_context(tc.tile_pool(name="hs", bufs=2))
        wk = ctx.enter_context(tc.tile_pool(name="wk", bufs=2))
        vpool = ctx.enter_context(tc.tile_pool(name="vp", bufs=2))
        lvl = ctx.enter_context(tc.tile_pool(name="lvl", bufs=2))
        ppool = ctx.enter_context(tc.tile_pool(name="pp", bufs=2, space="PSUM"))
        rpool = ctx.enter_context(tc.tile_pool(name="rp", bufs=2, space="PSUM"))
        outp = ctx.enter_context(tc.tile_pool(name="op", bufs=1))

        b1t = const.tile([TILE, D1], F16)
        nc.sync.dma_start(out=b1t[:], in_=b1r[:])
        w2t = const.tile([D1, 4], F16)
        nc.sync.dma_start(out=w2t[:], in_=w2p[:])
        ident = const.tile([TILE, TILE], F16)
        make_identity(nc, ident[:])

        if repeat:
            ctx.enter_context(tc.For_i(0, repeat, 1))
        o1a16 = outp.tile([TILE, D1, ntiles], F16, tag="o1a16")
        sa16 = outp.tile([TILE, NH, ntiles], F16, tag="sa16")

        st = {}
        pairs = [groups[i : i + 2] for i in range(0, len(groups), 2)]
        gidx = 0
        for pair in pairs:
            ids = list(range(gidx, gidx + len(pair)))
            gidx += len(pair)
            for gi, (t0, ng, dg) in zip(ids, pair):
                L = ng * dg
                hst = hspool.tile([TILE, 36, lgm], F16, tag="hst")
                nc.sync.dma_start(
                    out=hst[:, :, 0:L],
                    in_=hsd[:, offs[gi] : offs[gi] + 36 * L].rearrange(
                        "p (c j) -> p c j", c=36
                    ),
                )
                st[gi] = {"hst": hst}
            for gi, (t0, ng, dg) in zip(ids, pair):
                L = ng * dg
                lg = wk.tile([TILE, NH, lgm], F16, tag="lg")
                nc.vector.tensor_tensor(
                    out=lg[:, :, 0:L],
                    in0=st[gi]["hst"][:, D1 : D1 + NH, 0:L],
                    in1=st[gi]["hst"][:, D1 + NH : REC, 0:L],
                    op=mybir.AluOpType.add,
                )
                st[gi]["lg"] = lg
            for gi, (t0, ng, dg) in zip(ids, pair):
                # e = exp(lrelu(x)) = max(exp(x), exp(0.2 x)): one DVE op less
                L = ng * dg
                e1 = wk.tile([TILE, NH, lgm], F16, tag="ls")
                nc.scalar.activation(
                    out=e1[:, :, 0:L], in_=st[gi]["lg"][:, :, 0:L],
                    func=mybir.ActivationFunctionType.Exp,
                )
                st[gi]["e1"] = e1
            for gi, (t0, ng, dg) in zip(ids, pair):
                L = ng * dg
                e2 = wk.tile([TILE, NH, lgm], F16, tag="e2")
                nc.scalar.activation(
                    out=e2[:, :, 0:L], in_=st[gi]["lg"][:, :, 0:L],
                    func=mybir.ActivationFunctionType.Exp, scale=NEG_SLOPE,
                )
                st[gi]["e2"] = e2
            for gi, (t0, ng, dg) in zip(ids, pair):
                L = ng * dg
                et = wk.tile([TILE, NH, lgm], F16, tag="et")
                nc.vector.tensor_tensor(
                    out=et[:, :, 0:L],
                    in0=st[gi]["e1"][:, :, 0:L],
                    in1=st[gi]["e2"][:, :, 0:L],
                    op=mybir.AluOpType.max,
                )
                st[gi]["et"] = et
            for gi, (t0, ng, dg) in zip(ids, pair):
                L = ng * dg
                V = vpool.tile([TILE, NH, CH, lgm], F16, tag="V")
                nc.vector.tensor_tensor(
                    out=V[:, :, :, 0:L],
                    in0=st[gi]["hst"][:, 0:D1, 0:L].rearrange(
                        "p (h c) j -> p h c j", h=NH
                    ),
                    in1=st[gi]["et"][:, :, 0:L]
                    .unsqueeze(2)
                    .to_broadcast([TILE, NH, CH, L]),
                    op=mybir.AluOpType.mult,
                )
                st[gi]["V"] = V
            for gi, (t0, ng, dg) in zip(ids, pair):
                # halving level 1: Vh[.., j] = V[.., j] + V[.., k+j]
                L = ng * dg
                k = dg // 2
                flat = lvl.tile([TILE, caph], F16, tag="lv")
                Vh = flat[:, 0 : D1 * ng * k].rearrange(
                    "p (c t j) -> p c t j", c=D1, t=ng
                )
                V5 = st[gi]["V"][:, :, :, 0:L].rearrange(
                    "p h c (t j) -> p (h c) t j", j=dg
                )
                nc.vector.tensor_tensor(
                    out=Vh[:, :, :, 0:k],
                    in0=V5[:, :, :, 0:k],
                    in1=V5[:, :, :, k : 2 * k],
                    op=mybir.AluOpType.add,
                )
                st[gi]["Vh"] = Vh
            for gi, (t0, ng, dg) in zip(ids, pair):
                # halving level 2
                k = dg // 2
                k2 = k // 2
                flat2 = lvl.tile([TILE, caph // 2 + D1], F16, tag="lv2")
                Vh2 = flat2[:, 0 : D1 * ng * k2].rearrange(
                    "p (c t j) -> p c t j", c=D1, t=ng
                )
                nc.vector.tensor_tensor(
                    out=Vh2[:, :, :, 0:k2],
                    in0=st[gi]["Vh"][:, :, :, 0:k2],
                    in1=st[gi]["Vh"][:, :, :, k2 : 2 * k2],
                    op=mybir.AluOpType.add,
                )
                st[gi]["Vh"] = Vh2
            with nc.allow_low_precision(reason="fp16 segment sums, 2e-2 gate"):
                for gi, (t0, ng, dg) in zip(ids, pair):
                    nc.vector.tensor_reduce(
                        out=o1a16[:, :, t0 : t0 + ng],
                        in_=st[gi]["Vh"],
                        axis=mybir.AxisListType.X,
                        op=mybir.AluOpType.add,
                    )
                for gi, (t0, ng, dg) in zip(ids, pair):
                    L = ng * dg
                    E5 = st[gi]["et"][:, :, 0:L].rearrange(
                        "p h (t j) -> p h t j", j=dg
                    )
                    nc.vector.tensor_reduce(
                        out=sa16[:, :, t0 : t0 + ng],
                        in_=E5,
                        axis=mybir.AxisListType.X,
                        op=mybir.AluOpType.add,
                    )
            st.clear()

        # ---- finishing: two tile-range halves, instruction-interleaved ----
        inv = outp.tile([TILE, NH, ntiles], F32, tag="inv")
        invh = outp.tile([TILE, NH, ntiles], F16, tag="invh")
        o1f = outp.tile([TILE, NH, CH, ntiles], F16, tag="o1f")
        e1 = outp.tile([TILE, NH, CH, ntiles], F16, tag="e1")

        def fin_chain(ts, te):
            w = te - ts
            yield lambda: nc.vector.tensor_scalar_add(
                out=inv[:, :, ts:te], in0=sa16[:, :, ts:te], scalar1=1e-16
            )
            yield lambda: nc.vector.reciprocal(
                out=inv[:, :, ts:te], in_=inv[:, :, ts:te]
            )
            yield lambda: nc.vector.tensor_copy(
                out=invh[:, :, ts:te], in_=inv[:, :, ts:te]
            )
            yield lambda: nc.vector.tensor_tensor(
                out=o1f[:, :, :, ts:te],
                in0=o1a16[:, :, ts:te].rearrange("p (h c) t -> p h c t", h=NH),
                in1=invh[:, :, ts:te]
                .unsqueeze(2)
                .to_broadcast([TILE, NH, CH, w]),
                op=mybir.AluOpType.mult,
            )
            yield lambda: nc.vector.tensor_tensor(
                out=o1f[:, :, :, ts:te],
                in0=o1f[:, :, :, ts:te],
                in1=b1t[:]
                .rearrange("p (h c) -> p h c", h=NH)
                .unsqueeze(-1)
                .to_broadcast([TILE, NH, CH, w]),
                op=mybir.AluOpType.add,
            )
            yield lambda: nc.vector.tensor_scalar_min(
                out=e1[:, :, :, ts:te], in0=o1f[:, :, :, ts:te], scalar1=0.0
            )
            yield lambda: nc.scalar.activation(
                out=e1[:, :, :, ts:te],
                in_=e1[:, :, :, ts:te],
                func=mybir.ActivationFunctionType.Exp,
            )
            yield lambda: nc.vector.tensor_scalar_add(
                out=e1[:, :, :, ts:te], in0=e1[:, :, :, ts:te], scalar1=-1.0
            )
            yield lambda: nc.vector.tensor_scalar_max(
                out=o1f[:, :, :, ts:te], in0=o1f[:, :, :, ts:te], scalar1=0.0
            )
            yield lambda: nc.vector.tensor_tensor(
                out=o1f[:, :, :, ts:te],
                in0=o1f[:, :, :, ts:te],
                in1=e1[:, :, :, ts:te],
                op=mybir.AluOpType.add,
            )

        for s in fin_chain(0, ntiles):
            s()

        # R2 = [h2 | a_src2 | a_dst2] = elu_out @ w2p via PE transposes
        o1tsb = outp.tile([D1, padn], F16, tag="o1t")
        pt = None
        for t in range(ntiles):
            q = t % 8
            if q == 0:
                pt = ppool.tile([D1, 8 * TILE], F16, tag="pt")
            nc.tensor.transpose(
                out=pt[:, q * TILE : (q + 1) * TILE],
                in_=o1f[:, :, :, t].rearrange("p h c -> p (h c)"),
                identity=ident[:],
            )
            if q == 7 or t == ntiles - 1:
                nc.vector.tensor_copy(
                    out=o1tsb[:, (t - q) * TILE : (t + 1) * TILE],
                    in_=pt[:, 0 : (q + 1) * TILE],
                )
        r2all = outp.tile([TILE, ntiles, 4], F16, tag="r2all")
        r2p = None
        for t in range(ntiles):
            q = t % 32
            if q == 0:
                r2p = rpool.tile([TILE, 32 * 4], F32, tag="r2p")
            nc.tensor.matmul(
                out=r2p[:, q * 4 : (q + 1) * 4],
                lhsT=o1tsb[:, t * TILE : (t + 1) * TILE],
                rhs=w2t[:],
                start=True,
                stop=True,
            )
            if q == 31 or t == ntiles - 1:
                nc.vector.tensor_copy(
                    out=r2all[:, t - q : t + 1, :],
                    in_=r2p[:, 0 : (q + 1) * 4].rearrange("p (t c) -> p t c", c=4),
                )
        nc.sync.dma_start(
            out=r2[:].rearrange("(t n) c -> n t c", n=TILE), in_=r2all[:]
        )
    return nc


# ------------------------------------------------------------- launch C (L2)


def _build_l2(Dt, groups, ntiles, padn, repeat=None):
    """Layer 2 (1 head, 2 ch) from grouped planar [h2(2) | a_src2 | a_dst2]
    slots, bias and log_softmax (channel-pair ops, no reduces in finishing).
    Group stages pair-interleaved as in _build_l1."""
    cb = 4 * sum(ng * dg for _, ng, dg in groups)
    lgm = max(ng * dg for _, ng, dg in groups)
    nc = bass.Bass("TRN2")
    xed = nc.declare_dram_parameter("xed", [TILE, cb], F16, isOutput=False)
    b2r = nc.declare_dram_parameter("b2r", [TILE, 2], F32, isOutput=False)
    y = nc.declare_dram_parameter("y", [padn, 2], F32, isOutput=True)

    offs = []
    off = 0
    for _, ng, dg in groups:
        offs.append(off)
        off += 4 * ng * dg

    with ExitStack() as ctx:
        tc = ctx.enter_context(tile.TileContext(nc))
        const = ctx.enter_context(tc.tile_pool(name="const", bufs=1))
        xp = ctx.enter_context(tc.tile_pool(name="xp", bufs=2))
        wk = ctx.enter_context(tc.tile_pool(name="wk", bufs=2))
        outp = ctx.enter_context(tc.tile_pool(name="op", bufs=1))

        b2t = const.tile([TILE, 2], F32)
        nc.sync.dma_start(out=b2t[:], in_=b2r[:])

        if repeat:
            ctx.enter_context(tc.For_i(0, repeat, 1))
        acc2 = outp.tile([TILE, 2, ntiles], F16, tag="acc2")
        s2h = outp.tile([TILE, ntiles], F16, tag="s2h")

        st = {}
        pairs = [groups[i : i + 2] for i in range(0, len(groups), 2)]
        gidx = 0
        for pair in pairs:
            ids = list(range(gidx, gidx + len(pair)))
            gidx += len(pair)
            for gi, (t0, ng, dg) in zip(ids, pair):
                L = ng * dg
                xe = xp.tile([TILE, 4, lgm], F16, tag="xe")
                nc.sync.dma_start(
                    out=xe[:, :, 0:L],
                    in_=xed[:, offs[gi] : offs[gi] + 4 * L].rearrange(
                        "p (c j) -> p c j", c=4
                    ),
                )
                st[gi] = {"xe": xe}
            for gi, (t0, ng, dg) in zip(ids, pair):
                L = ng * dg
                lg = wk.tile([TILE, lgm], F16, tag="lg")
                nc.vector.tensor_tensor(
                    out=lg[:, 0:L], in0=st[gi]["xe"][:, 2, 0:L],
                    in1=st[gi]["xe"][:, 3, 0:L], op=mybir.AluOpType.add,
                )
                st[gi]["lg"] = lg
            for gi, (t0, ng, dg) in zip(ids, pair):
                L = ng * dg
                e1 = wk.tile([TILE, lgm], F16, tag="ls")
                nc.scalar.activation(
                    out=e1[:, 0:L], in_=st[gi]["lg"][:, 0:L],
                    func=mybir.ActivationFunctionType.Exp,
                )
                st[gi]["e1"] = e1
            for gi, (t0, ng, dg) in zip(ids, pair):
                L = ng * dg
                e2 = wk.tile([TILE, lgm], F16, tag="e2")
                nc.scalar.activation(
                    out=e2[:, 0:L], in_=st[gi]["lg"][:, 0:L],
                    func=mybir.ActivationFunctionType.Exp, scale=NEG_SLOPE,
                )
                st[gi]["e2"] = e2
            for gi, (t0, ng, dg) in zip(ids, pair):
                L = ng * dg
                et = wk.tile([TILE, lgm], F16, tag="et")
                nc.vector.tensor_tensor(
                    out=et[:, 0:L], in0=st[gi]["e1"][:, 0:L],
                    in1=st[gi]["e2"][:, 0:L], op=mybir.AluOpType.max,
                )
                st[gi]["et"] = et
            for gi, (t0, ng, dg) in zip(ids, pair):
                L = ng * dg
                V = wk.tile([TILE, 2, lgm], F16, tag="V")
                nc.vector.tensor_tensor(
                    out=V[:, :, 0:L],
                    in0=st[gi]["xe"][:, 0:2, 0:L],
                    in1=st[gi]["et"][:, 0:L]
                    .unsqueeze(1)
                    .to_broadcast([TILE, 2, L]),
                    op=mybir.AluOpType.mult,
                )
                st[gi]["V"] = V
            with nc.allow_low_precision(reason="fp16 segment sums, 2e-2 gate"):
                for gi, (t0, ng, dg) in zip(ids, pair):
                    L = ng * dg
                    V5 = st[gi]["V"][:, :, 0:L].rearrange(
                        "p c (t j) -> p c t j", j=dg
                    )
                    nc.vector.tensor_reduce(
                        out=acc2[:, :, t0 : t0 + ng],
                        in_=V5,
                        axis=mybir.AxisListType.X,
                        op=mybir.AluOpType.add,
                    )
                for gi, (t0, ng, dg) in zip(ids, pair):
                    L = ng * dg
                    E5 = st[gi]["et"][:, 0:L].rearrange(
                        "p (t j) -> p t j", j=dg
                    )
                    nc.vector.tensor_reduce(
                        out=s2h[:, t0 : t0 + ng],
                        in_=E5,
                        axis=mybir.AxisListType.X,
                        op=mybir.AluOpType.add,
                    )
            st.clear()

        # ---- finishing: two tile-range halves, instruction-interleaved ----
        inv = outp.tile([TILE, ntiles], F32, tag="inv")
        z = outp.tile([TILE, 2, ntiles], F32, tag="z")
        m = outp.tile([TILE, ntiles], F32, tag="m")
        ez = outp.tile([TILE, 2, ntiles], F32, tag="ez")
        ss = outp.tile([TILE, ntiles], F32, tag="ss")
        yt = outp.tile([TILE, 2, ntiles], F32, tag="yt")
        yt2 = outp.tile([TILE, ntiles, 2], F32, tag="yt2")

        def fin_chain(ts, te):
            w = te - ts
            yield lambda: nc.vector.tensor_scalar_add(
                out=inv[:, ts:te], in0=s2h[:, ts:te], scalar1=1e-16
            )
            yield lambda: nc.vector.reciprocal(
                out=inv[:, ts:te], in_=inv[:, ts:te]
            )
            yield lambda: nc.vector.tensor_tensor(
                out=z[:, :, ts:te],
                in0=acc2[:, :, ts:te],
                in1=inv[:, ts:te].unsqueeze(1).to_broadcast([TILE, 2, w]),
                op=mybir.AluOpType.mult,
            )
            yield lambda: nc.vector.tensor_tensor(
                out=z[:, :, ts:te],
                in0=z[:, :, ts:te],
                in1=b2t[:].unsqueeze(-1).to_broadcast([TILE, 2, w]),
                op=mybir.AluOpType.add,
            )
            yield lambda: nc.vector.tensor_tensor(
                out=m[:, ts:te], in0=z[:, 0, ts:te], in1=z[:, 1, ts:te],
                op=mybir.AluOpType.max,
            )
            yield lambda: nc.vector.tensor_tensor(
                out=z[:, :, ts:te],
                in0=z[:, :, ts:te],
                in1=m[:, ts:te].unsqueeze(1).to_broadcast([TILE, 2, w]),
                op=mybir.AluOpType.subtract,
            )
            yield lambda: nc.scalar.activation(
                out=ez[:, :, ts:te], in_=z[:, :, ts:te],
                func=mybir.ActivationFunctionType.Exp,
            )
            yield lambda: nc.vector.tensor_tensor(
                out=ss[:, ts:te], in0=ez[:, 0, ts:te], in1=ez[:, 1, ts:te],
                op=mybir.AluOpType.add,
            )
            yield lambda: nc.scalar.activation(
                out=ss[:, ts:te], in_=ss[:, ts:te],
                func=mybir.ActivationFunctionType.Ln,
            )
            yield lambda: nc.vector.tensor_tensor(
                out=yt[:, :, ts:te],
                in0=z[:, :, ts:te],
                in1=ss[:, ts:te].unsqueeze(1).to_broadcast([TILE, 2, w]),
                op=mybir.AluOpType.subtract,
            )
            yield lambda: nc.vector.tensor_copy(
                out=yt2[:, ts:te, :],
                in_=yt[:, :, ts:te].rearrange("p c t -> p t c"),
            )

        for s in fin_chain(0, ntiles):
            s()
        nc.sync.dma_start(
            out=y[:].rearrange("(t n) c -> n t c", n=TILE), in_=yt2[:]
        )
    return nc


# ------------------------------------------------------------------- driver


def _run_gat(x, edge_index, W1, att_src1, att_dst1, b1, W2, att_src2, att_dst2, b2,
             n_cores=NC, timing=None):
    n_nodes, fdim = x.shape
    nh, ch = att_src1.shape

    src = np.concatenate([np.asarray(edge_index[0]), np.arange(n_nodes)]).astype(
        np.int64
    )
    dst = np.concatenate([np.asarray(edge_index[1]), np.arange(n_nodes)]).astype(
        np.int64
    )

    per, ntiles, padn, Dt, nblocks, slot_src, orders = _plan(
        src, dst, n_nodes, n_cores
    )
    groups = _groups(Dt)

    W1 = np.asarray(W1, np.float32)
    att_src1 = np.asarray(att_src1, np.float32)
    att_dst1 = np.asarray(att_dst1, np.float32)
    W2 = np.asarray(W2, np.float32)
    att_src2 = np.asarray(att_src2, np.float32)
    att_dst2 = np.asarray(att_dst2, np.float32)

    # fused weights
    w_asrc1 = np.stack(
        [W1[:, h * ch : (h + 1) * ch] @ att_src1[h] for h in range(nh)], axis=1
    )  # [F, nh]
    w_adst1 = np.stack(
        [W1[:, h * ch : (h + 1) * ch] @ att_dst1[h] for h in range(nh)], axis=1
    )
    w1p = np.concatenate([W1, w_asrc1, w_adst1], axis=1).astype(NP16)  # [F, REC]
    w_asrc2 = W2 @ att_src2[0]
    w_adst2 = W2 @ att_dst2[0]
    w2p = np.concatenate(
        [W2, w_asrc2[:, None], w_adst2[:, None]], axis=1
    ).astype(NP16)  # [D1, 4]

    x = np.asarray(x, np.float32)

    # ---- launch A: per-node R1 ----
    in_maps0 = []
    for c in range(n_cores):
        ids = orders[c]
        real = ids >= 0
        xs = np.where(real[:, None], x[np.maximum(ids, 0)], 0.0)  # [padn, F]
        in_maps0.append(
            {"xt": np.ascontiguousarray(xs.T.astype(NP16)), "w1p": w1p}
        )
    nc0 = _build_a(padn, fdim)
    _split_waits(nc0)
    import time as _time

    t0 = _time.perf_counter()
    res0 = run_bass_kernel_spmd(nc0, in_maps0, list(range(n_cores)))
    t1 = _time.perf_counter()
    if timing is not None:
        timing["a_first_s"] = t1 - t0
        timing["nc0"] = nc0
        timing["in_maps0"] = in_maps0

    # R1 lookup table: [h1(32) | a_src(2) | a_dst(2)], pad row kills e
    r1tab = np.zeros((n_nodes + 1, REC), NP16)
    r1tab[n_nodes, D1 : D1 + NH] = BIG_NEG
    for c in range(n_cores):
        ids = orders[c]
        real = ids >= 0
        r1tab[ids[real]] = res0.results[c]["r1"][:, real].T

    pslots = [
        _padded_slots(slot_src[c], Dt, groups, n_nodes) for c in range(n_cores)
    ]
    groups2 = _groups(Dt, L2_BUDGET)
    pslots2 = [
        _padded_slots(slot_src[c], Dt, groups2, n_nodes) for c in range(n_cores)
    ]

    # ---- launch B inputs: grouped channel-major slot planes ----
    in_maps1 = []
    b1r = np.broadcast_to(np.asarray(b1, NP16), (TILE, D1)).copy()
    for c in range(n_cores):
        chunks = []
        for gi, (t0g, ng, dg) in enumerate(groups):
            g = r1tab[pslots[c][gi]]  # [L, TILE, REC]
            gt = g.transpose(1, 2, 0)  # [TILE, REC, L]
            hp = gt[:, 0:D1, :]
            asr = gt[:, D1 : D1 + NH, :]
            ids = orders[c][t0g * TILE : (t0g + ng) * TILE].reshape(ng, TILE)
            adv = r1tab[np.maximum(ids, 0), D1 + NH : REC]  # [ng, TILE, NH]
            adv = adv * (ids >= 0)[:, :, None].astype(NP16)
            ade = np.repeat(adv, dg, axis=0)  # [L, TILE, NH]
            ade = ade.transpose(1, 2, 0)  # [TILE, NH, L]
            chunks.append(
                np.concatenate([hp, asr, ade], axis=1).reshape(TILE, 36 * ng * dg)
            )
        in_maps1.append(
            {
                "hsd": np.ascontiguousarray(np.concatenate(chunks, axis=1)),
                "b1r": b1r,
                "w2p": w2p,
            }
        )

    nc1 = _build_l1(Dt, groups, ntiles, padn)
    _split_waits(nc1)
    t2 = _time.perf_counter()
    res1 = run_bass_kernel_spmd(nc1, in_maps1, list(range(n_cores)))
    t3 = _time.perf_counter()
    if timing is not None:
        timing["l1_first_s"] = t3 - t2
        timing["nc1"] = nc1
        timing["in_maps1"] = in_maps1

    # R2 lookup table: [h2(2) | a_src2 | a_dst2]
    r2tab = np.zeros((n_nodes + 1, 4), NP16)
    r2tab[n_nodes, 2] = BIG_NEG
    for c in range(n_cores):
        ids = orders[c]
        real = ids >= 0
        r2tab[ids[real]] = res1.results[c]["r2"][real]

    # ---- launch C inputs: grouped planar slots ----
    in_maps2 = []
    b2r = np.broadcast_to(np.asarray(b2, np.float32), (TILE, 2)).copy()
    for c in range(n_cores):
        chunks = []
        for gi, (t0g, ng, dg) in enumerate(groups2):
            g = r2tab[pslots2[c][gi]]  # [L, TILE, 4]
            gt = g.transpose(1, 2, 0)  # [TILE, 4, L]
            h2 = gt[:, 0:2, :]
            as2 = gt[:, 2:3, :]
            ids = orders[c][t0g * TILE : (t0g + ng) * TILE].reshape(ng, TILE)
            adv = r2tab[np.maximum(ids, 0), 3] * (ids >= 0).astype(NP16)
            ad2 = np.repeat(adv, dg, axis=0).T[:, None, :]  # [TILE, 1, L]
            chunks.append(
                np.concatenate([h2, as2, ad2], axis=1).reshape(TILE, 4 * ng * dg)
            )
        in_maps2.append(
            {
                "xed": np.ascontiguousarray(np.concatenate(chunks, axis=1)),
                "b2r": b2r,
            }
        )

    nc2 = _build_l2(Dt, groups2, ntiles, padn)
    _split_waits(nc2)
    t4 = _time.perf_counter()
    res2 = run_bass_kernel_spmd(nc2, in_maps2, list(range(n_cores)))
    t5 = _time.perf_counter()
    if timing is not None:
        timing["l2_first_s"] = t5 - t4
        timing["nc2"] = nc2
        timing["in_maps2"] = in_maps2

    out = np.zeros((n_nodes, 2), np.float32)
    for c in range(n_cores):
        yc = res2.results[c]["y"]
        ids = orders[c]
        real = ids >= 0
        out[ids[real]] = yc[real]
    return out


def kernel(x, edge_index, W1, att_src1, att_dst1, b1, W2, att_src2, att_dst2, b2):
    return _run_gat(
        np.asarray(x, np.float32),
        np.asarray(edge_index),
        W1,
        att_src1,
        att_dst1,
        b1,
        W2,
        att_src2,
        att_dst2,
        b2,
    )
